# revision 45
# baseline (speedup 1.0000x reference)
"""Trainium2 Bass kernel for nn_Attention_xxc (dense transformer attention
with hop-distance bias). Data-parallel over batch: 8 cores x 2 batches.

Wire-traffic-minimized design: the warm end-to-end latency of this problem
is dominated by host<->device transfer over the axon tunnel (~50 MB/s), so
every shared tensor is shipped sharded 1/8-per-core and AllGathered on
device over NeuronLink; the hop-bias mixture  alpha_h * sum_k w_hk Hstack_k
is never materialized on the host - the PE folds it into the score matmuls
as  S.T = K^T Q + sum_k (c_hk I) @ Hstack_k.T  accumulated in PSUM.

Per-core layout (core c of 8):
  - xn [2048, 512] float8_e4m3: the core's own 2 batches, natural layout;
    cast to bf16 and PE-transposed on device via identity matmuls.
  - shr [1061, 1024] u8: the core's 1/8 shard of ONE packed shared blob
    (wqkvT bf16 with q pre-scaled | wprojT bf16 | 40 scaled identities
    c_hk*I bf16 | eye bf16 | bproj bf16 | Hstack_k^T as u8 in [0,255]);
    a single device AllGather rebuilds the full blob, sub-tensors are
    read out with bitcast/rearranged APs.
  - qkv: q,k TRANSPOSED ([outch, tok] bf16), v NATURAL with a ones column
    per head (65 cols/head) so the AV matmul also produces the softmax
    denominator in row 64.
  - output y [2048, 512] bf16, host casts to f32.
Runner: persistent jax jit of the bass_exec custom call (no per-call
retrace); donated output buffers reuse the previous call's device output
buffers (first call builds zeros on device — no host-side zero upload);
the async fp8-x device_put overlaps the host-side blob build.
Error budget (validated vs reference on the fixed seed): fp8 x -> median
rel err 0.0091, scale-relative absmax 0.0115, under the 2e-2 gate.
"""
import sys

sys.path.insert(0, "/opt/trn_rl_repo")

import numpy as np
import ml_dtypes

B, N, DIM = 16, 1024, 512
H, HD, KH = 8, 64, 5
SCALE = HD ** -0.5
NCORES = 8
BPC = B // NCORES          # batches per core
TOK = BPC * N              # tokens per core = 2048
HTR = KH * N               # 5120 rows of flat transposed-Hstack
CER = H * KH * 128         # 5120 rows of flat scaled-identity stack

# shared-blob layout, in rows of 1024 bytes (= 512 bf16 / 1024 u8):
#   wqkv bf16 [512,1536] | wproj bf16 [512,512] | ceye bf16 [5120,128]
#   | eye bf16 [128,128] | bproj bf16 [512] | hts u8 [5120,1024] | pad
R_WQKV = 0
R_WPROJ = R_WQKV + 512 * 3
R_CEYE = R_WPROJ + 512
R_EYE = R_CEYE + CER // 4
R_BPROJ = R_EYE + 32
R_HTS = R_BPROJ + 1
SHR_ROWS = -(-(R_HTS + HTR) // 8) * 8    # pad to a multiple of 8 cores

_CACHE = {}


def _build():
    import concourse.bass as bass
    import concourse.bacc as bacc
    import concourse.mybir as mybir
    from concourse.tile import TileContext

    f32 = mybir.dt.float32
    bf16 = mybir.dt.bfloat16
    u8 = mybir.dt.uint8
    f8 = mybir.dt.float8e4
    EXP = mybir.ActivationFunctionType.Exp
    MUL = mybir.AluOpType.mult
    ADD = mybir.AluOpType.add
    BYP = mybir.AluOpType.bypass
    RG = [list(range(NCORES))]

    nc = bacc.Bacc(num_devices=NCORES)
    xn = nc.declare_dram_parameter("xn", [TOK, DIM], f8, isOutput=False)
    shr_in = nc.declare_dram_parameter("shr_in", [SHR_ROWS // 8, 1024], u8, isOutput=False)
    y = nc.declare_dram_parameter("y", [TOK, DIM], bf16, isOutput=True)

    NT = TOK // 128            # 16 token tiles per core
    VW = H * (HD + 1)          # 520: v row width with ones col per head

    with TileContext(nc) as tc:
        with (
            tc.tile_pool(name="dram", bufs=1, space="DRAM") as DR,
            tc.tile_pool(name="qk", bufs=1) as QK,
            tc.tile_pool(name="vres", bufs=1) as VR,
            tc.tile_pool(name="wp", bufs=1) as WP,
            tc.tile_pool(name="outT", bufs=1) as OT,
            tc.tile_pool(name="const", bufs=1) as CONST,
        ):
            # ---------------- phase 0: AllGather the one shared blob ----------------
            bnc = DR.tile([SHR_ROWS // 8, 1024], u8, tag="b_shr", name="b_shr")
            shr_full = DR.tile([SHR_ROWS, 1024], u8, tag="g_shr", name="g_shr")
            nc.gpsimd.dma_start(bnc[:], shr_in[:])
            nc.gpsimd.collective_compute(
                "AllGather", BYP, replica_groups=RG,
                ins=[bnc.opt()], outs=[shr_full.opt()])

            eye_t = CONST.tile([128, 128], bf16, tag="eye", name="eye")
            nc.sync.dma_start(
                out=eye_t[:],
                in_=shr_full[R_EYE: R_EYE + 32, :].bitcast(bf16)
                .rearrange("a (b c) -> (a b) c", b=4))
            ones_t = CONST.tile([1, 128], bf16, tag="ones", name="ones")
            nc.vector.memset(ones_t[:], 1.0)
            ceye_t = CONST.tile([128, H * KH * 128], bf16, tag="ceye", name="ceye")
            for j in range(H * KH):
                nc.sync.dma_start(
                    out=ceye_t[:, j * 128:(j + 1) * 128],
                    in_=shr_full[R_CEYE + 32 * j: R_CEYE + 32 * (j + 1), :]
                    .bitcast(bf16).rearrange("a (b c) -> (a b) c", b=4))
            wp_t = [WP.tile([128, DIM], bf16, tag=f"wp{c}", name=f"wp{c}") for c in range(4)]
            for c in range(4):
                nc.sync.dma_start(
                    out=wp_t[c][:],
                    in_=shr_full[R_WPROJ + c * 128: R_WPROJ + (c + 1) * 128, :]
                    .bitcast(bf16))

            qk_t = [QK.tile([128, TOK], bf16, tag=f"qk{o}", name=f"qk{o}") for o in range(8)]
            v_t = [VR.tile([128, VW], bf16, tag=f"v{t}", name=f"v{t}") for t in range(NT)]
            oT_t = [OT.tile([128, N], bf16, tag=f"oT{b}_{c}", name=f"oT{b}_{c}")
                    for b in range(BPC) for c in range(4)]

            # broadcast bproj across 128 partitions: ones^T [128] x bproj [1,512]
            bpb_t = CONST.tile([128, DIM], f32, tag="bpb", name="bpb")
            bpr_t = CONST.tile([1, DIM], bf16, tag="bpr", name="bpr")
            nc.sync.dma_start(out=bpr_t[:],
                              in_=shr_full[R_BPROJ: R_BPROJ + 1, :].bitcast(bf16))

            # ---------------- phase 1: x transpose + qkv projections ----------------
            with (
                tc.tile_pool(name="xw", bufs=1) as XW,
                tc.tile_pool(name="ps1", bufs=4, space="PSUM") as PS1,
                tc.tile_pool(name="pst", bufs=4, space="PSUM") as PST,
            ):
                psb = PS1.tile([128, DIM], f32, tag="ps1", name="ps1")
                nc.tensor.matmul(psb[:], ones_t[:], bpr_t[:], start=True, stop=True)
                nc.vector.tensor_copy(bpb_t[:], psb[:])

                xn_t = [XW.tile([128, DIM], bf16, tag=f"xn{t}", name=f"xn{t}")
                        for t in range(NT)]
                for t in range(NT):
                    x8 = XW.tile([128, DIM], f8, tag=f"x8_{t}", name=f"x8_{t}")
                    nc.sync.dma_start(out=x8[:], in_=xn[t * 128:(t + 1) * 128, :])
                    nc.vector.tensor_copy(xn_t[t][:], x8[:])
                xT_t = [XW.tile([128, TOK], bf16, tag=f"x{c}", name=f"x{c}") for c in range(4)]
                for t in range(NT):
                    for c in range(4):
                        pst = PST.tile([128, 128], f32, tag="pst", name="pst")
                        nc.tensor.matmul(pst[:], xn_t[t][:, c * 128:(c + 1) * 128],
                                         eye_t[:], start=True, stop=True)
                        nc.vector.tensor_copy(xT_t[c][:, t * 128:(t + 1) * 128], pst[:])

                wq_t = [XW.tile([128, 3 * DIM], bf16, tag=f"w{c}", name=f"w{c}") for c in range(4)]
                for c in range(4):
                    for t in range(3):
                        nc.sync.dma_start(
                            out=wq_t[c][:, 512 * t:512 * (t + 1)],
                            in_=shr_full[R_WQKV + 384 * c + t:
                                         R_WQKV + 384 * (c + 1): 3, :].bitcast(bf16))

                # q,k transposed: qkvT[o_tile, tok] ; o tiles 0..7 cover q,k
                for o in range(8):
                    for t in range(4):           # tok chunks of 512
                        ps = PS1.tile([128, 512], f32, tag="ps1", name="ps1")
                        for c in range(4):
                            nc.tensor.matmul(
                                ps[:], wq_t[c][:, o * 128:(o + 1) * 128],
                                xT_t[c][:, t * 512:(t + 1) * 512],
                                start=(c == 0), stop=(c == 3))
                        nc.vector.tensor_copy(qk_t[o][:, t * 512:(t + 1) * 512], ps[:])
                # v natural: [tok_tile, vch] -> packed per head with ones col
                for t in range(NT):
                    ps = PS1.tile([128, 512], f32, tag="ps1", name="ps1")
                    for c in range(4):
                        nc.tensor.matmul(
                            ps[:], xT_t[c][:, t * 128:(t + 1) * 128],
                            wq_t[c][:, 2 * DIM:3 * DIM],
                            start=(c == 0), stop=(c == 3))
                    dst = v_t[t][:, 0:VW].rearrange("p (h s) -> p h s", s=HD + 1)
                    nc.vector.tensor_copy(
                        dst[:, :, 0:HD],
                        ps[:].rearrange("p (h s) -> p h s", s=HD))
                    nc.vector.memset(dst[:, :, HD:HD + 1], 1.0)

            # ---------------- phase 2: attention ----------------
            with (
                tc.tile_pool(name="htu", bufs=3) as HTU,
                tc.tile_pool(name="htp", bufs=4) as HTP,
                tc.tile_pool(name="pp", bufs=18) as PP,
                tc.tile_pool(name="nrm", bufs=4) as NRM,
                tc.tile_pool(name="ysb", bufs=3) as YSB,
                tc.tile_pool(name="pss", bufs=2, space="PSUM") as PSS,
                tc.tile_pool(name="pso", bufs=1, space="PSUM") as PSO,
                tc.tile_pool(name="psm", bufs=2, space="PSUM") as PSM,
            ):
                for h in range(H):
                    qt, po = qk_t[h // 2], (h % 2) * 64
                    kt = qk_t[4 + h // 2]
                    p_tiles = [[], []]
                    for mi in range(8):
                        hu = HTU.tile([128, KH * N], u8, tag="hu", name="hu")
                        for k in range(KH):
                            nc.sync.dma_start(
                                out=hu[:, k * N:(k + 1) * N],
                                in_=shr_full[R_HTS + k * N + mi * 128:
                                             R_HTS + k * N + (mi + 1) * 128, :])
                        ht = HTP.tile([128, KH * N], bf16, tag="ht", name="ht")
                        nc.vector.tensor_copy(ht[:], hu[:])
                        for b in range(BPC):
                            t0 = b * N
                            ps = PSS.tile([128, N], f32, tag="pss", name="pss")
                            for nchunk in range(2):
                                sl = slice(nchunk * 512, (nchunk + 1) * 512)
                                nc.tensor.matmul(
                                    ps[:, sl],
                                    kt[po:po + 64, t0 + mi * 128: t0 + (mi + 1) * 128],
                                    qt[po:po + 64, t0 + nchunk * 512: t0 + (nchunk + 1) * 512],
                                    start=True, stop=False)
                                for k in range(KH):
                                    ci = (h * KH + k) * 128
                                    nc.tensor.matmul(
                                        ps[:, sl],
                                        ceye_t[:, ci:ci + 128],
                                        ht[:, k * N + nchunk * 512:
                                           k * N + (nchunk + 1) * 512],
                                        start=False, stop=(k == KH - 1))
                            pt = PP.tile([128, N], bf16, tag="p", name="p")
                            nc.scalar.activation(pt[:], ps[:], EXP)
                            p_tiles[b].append(pt)
                    for b in range(BPC):
                        pso = PSO.tile([HD + 1, N], f32, tag="pso", name="pso")
                        for mi in range(8):
                            for nchunk in range(2):
                                sl = slice(nchunk * 512, (nchunk + 1) * 512)
                                nc.tensor.matmul(
                                    pso[:, sl],
                                    v_t[b * 8 + mi][:, h * (HD + 1):(h + 1) * (HD + 1)],
                                    p_tiles[b][mi][:, sl],
                                    start=(mi == 0), stop=(mi == 7))
                        # denominator -> broadcast -> reciprocal -> normalize
                        d_t = NRM.tile([1, N], bf16, tag="d", name="d")
                        nc.vector.tensor_copy(d_t[:], pso[64:65, :])
                        R_t = NRM.tile([64, N], f32, tag="R", name="R")
                        for nchunk in range(2):
                            sl = slice(nchunk * 512, (nchunk + 1) * 512)
                            psr = PSM.tile([64, 512], f32, tag="psm", name="psm")
                            nc.tensor.matmul(psr[:], ones_t[:, 0:64], d_t[:, sl],
                                             start=True, stop=True)
                            nc.vector.reciprocal(R_t[:, sl], psr[:])
                        nc.vector.tensor_tensor(
                            oT_t[b * 4 + h // 2][po:po + 64, :],
                            pso[0:64, :], R_t[:], MUL)
                # ---------------- phase 3: output projection ----------------
                for b in range(BPC):
                    for t in range(8):
                        psy = PSM.tile([128, 512], f32, tag="psm", name="psm")
                        for c in range(4):
                            nc.tensor.matmul(
                                psy[:],
                                oT_t[b * 4 + c][:, t * 128:(t + 1) * 128],
                                wp_t[c][:], start=(c == 0), stop=(c == 3))
                        yt = YSB.tile([128, DIM], bf16, tag="y", name="y")
                        nc.vector.tensor_tensor(yt[:], psy[:], bpb_t[:], ADD)
                        nc.sync.dma_start(
                            out=y[b * N + t * 128: b * N + (t + 1) * 128, :],
                            in_=yt[:])
    nc.compile()
    return nc


def _prep_shared(Hstack, hop_logits_attn, rel_alpha, Wqkv, Wproj, bproj):
    """Build the shared blob [SHR_ROWS, 1024] u8 (concat-over-cores layout =
    the flat blob itself, so per-core shards are just row slices)."""
    bf = ml_dtypes.bfloat16
    lg = hop_logits_attn - hop_logits_attn.max(-1, keepdims=True)
    w = np.exp(lg)
    w /= w.sum(-1, keepdims=True)                      # [H, KH]
    # Hstack ships as uint8 (values in [0,1], quantization err ~ bf16's);
    # the 1/255 dequant scale is folded into the scaled identities.
    c_hk = (rel_alpha[:, None] * w).astype(np.float32) / 255.0  # [H, KH]
    eye = np.eye(128, dtype=np.float32)
    ceye = (c_hk.reshape(H * KH, 1, 1) * eye).astype(bf).reshape(CER, 128)
    shr = np.empty((SHR_ROWS, 1024), np.uint8)
    hdst = shr[R_HTS:R_HTS + HTR].reshape(KH, N, N)

    def quant(k):
        np.multiply(Hstack[k].T, 255.0, out=_QBUF[k])
        _QBUF[k] += 0.5
        hdst[k][:] = _QBUF[k]

    list(_POOL.map(quant, range(KH)))
    wqkvT = np.ascontiguousarray(Wqkv.T).astype(np.float32)
    wqkvT[:, :DIM] *= SCALE                            # fold q scaling
    u8row = lambda a: np.ascontiguousarray(a).view(np.uint8).reshape(-1, 1024)
    shr[R_WQKV:R_WPROJ] = u8row(wqkvT.astype(bf))
    shr[R_WPROJ:R_CEYE] = u8row(np.ascontiguousarray(Wproj.T).astype(bf))
    shr[R_CEYE:R_EYE] = u8row(ceye)
    shr[R_EYE:R_BPROJ] = u8row(eye.astype(bf))
    shr[R_BPROJ:R_HTS] = u8row(bproj.astype(np.float32).astype(bf)[None, :])
    shr[R_HTS + HTR:] = 0
    return shr


_QBUF = np.empty((KH, N, N), np.float32)

from concurrent.futures import ThreadPoolExecutor as _TPE
_POOL = _TPE(8)


def _cast_f8(x):
    """Multithreaded f32 -> float8_e4m3 cast (ml_dtypes cast is slow)."""
    out = np.empty(x.shape, ml_dtypes.float8_e4m3)
    chunks = np.array_split(np.arange(x.shape[0]), 8)

    def do(idx):
        out[idx[0]:idx[-1] + 1] = x[idx[0]:idx[-1] + 1]

    list(_POOL.map(do, chunks))
    return out


def _make_runner(nc):
    """Persistent-jit runner for the bass_exec custom call (the axon/PJRT
    path), so warm calls skip tracing and output zero-buffers are created
    on device instead of being shipped from the host."""
    import jax
    import jax.numpy as jnp
    from jax.sharding import Mesh, PartitionSpec, NamedSharding
    from jax.experimental.shard_map import shard_map
    from concourse import mybir
    from concourse.bass2jax import (
        _bass_exec_p, partition_id_tensor, install_neuronx_cc_hook)

    install_neuronx_cc_hook()
    partition_name = nc.partition_id_tensor.name if nc.partition_id_tensor else None
    in_names, out_names, out_avals = [], [], []
    for alloc in nc.m.functions[0].allocations:
        if not isinstance(alloc, mybir.MemoryLocationSet):
            continue
        name = alloc.memorylocations[0].name
        if alloc.kind == "ExternalInput":
            if name != partition_name:
                in_names.append(name)
        elif alloc.kind == "ExternalOutput":
            out_names.append(name)
            out_avals.append(jax.core.ShapedArray(
                tuple(alloc.tensor_shape), mybir.dt.np(alloc.dtype)))
    n_params = len(in_names)
    n_outs = len(out_avals)
    all_names = in_names + out_names
    if partition_name is not None:
        all_names = all_names + [partition_name]
    donate = tuple(range(n_params, n_params + n_outs))

    def _body(*args):
        operands = list(args)
        if partition_name is not None:
            operands.append(partition_id_tensor())
        outs = _bass_exec_p.bind(
            *operands, out_avals=tuple(out_avals), in_names=tuple(all_names),
            out_names=tuple(out_names), lowering_input_output_aliases=(),
            sim_require_finite=True, sim_require_nnan=True, nc=nc)
        return tuple(outs)

    devices = jax.devices()[:NCORES]
    mesh = Mesh(np.asarray(devices), ("core",))
    spec = NamedSharding(mesh, PartitionSpec("core"))
    in_specs = (PartitionSpec("core"),) * (n_params + n_outs)
    out_specs = (PartitionSpec("core"),) * n_outs
    sharded = jax.jit(
        shard_map(_body, mesh=mesh, in_specs=in_specs, out_specs=out_specs,
                  check_rep=False),
        donate_argnums=donate, keep_unused=True)

    zero_shapes = [(NCORES * a.shape[0], *a.shape[1:]) for a in out_avals]
    zero_dtypes = [a.dtype for a in out_avals]
    zeros_fn = jax.jit(
        lambda: tuple(jnp.zeros(s, d) for s, d in zip(zero_shapes, zero_dtypes)),
        out_shardings=tuple(spec for _ in out_avals))

    prev = []

    def run(global_in: dict):
        ins = [global_in[name] for name in in_names]
        # donate the previous call's (already fetched) output buffers as the
        # custom call's result allocation; first call builds zeros on device
        zs = tuple(prev) if prev else zeros_fn()
        prev.clear()
        outs = sharded(*ins, *zs)
        res = {name: np.asarray(o) for name, o in zip(out_names, outs)}
        prev.extend(outs)
        return res

    run.spec = spec
    return run


def kernel(**inputs):
    if "run" not in _CACHE:
        _CACHE["nc"] = _build()
        _CACHE["run"] = _make_runner(_CACHE["nc"])
    import jax
    run = _CACHE["run"]
    # cast + start the async x upload first; build the shared blob while the
    # 8.4MB of fp8 x streams over the tunnel
    x_f8 = _cast_f8(np.asarray(inputs["x"], np.float32).reshape(
        NCORES * TOK, DIM))
    x_dev = jax.device_put(x_f8, run.spec)
    shr = _prep_shared(
        np.asarray(inputs["Hstack"], np.float32),
        np.asarray(inputs["hop_logits_attn"], np.float32),
        np.asarray(inputs["rel_alpha"], np.float32),
        np.asarray(inputs["Wqkv"], np.float32),
        np.asarray(inputs["Wproj"], np.float32),
        np.asarray(inputs["bproj"], np.float32))
    outs = run({"xn": x_dev, "shr_in": shr})
    return outs["y"].astype(np.float32).reshape(B, N, DIM)


# revision 47
# speedup vs baseline: 1.1385x; 1.1385x over previous
"""Trainium2 Bass kernel for nn_Attention_xxc (dense transformer attention
with hop-distance bias). Data-parallel over batch: 8 cores x 2 batches.

Wire-traffic-minimized design: the warm end-to-end latency of this problem
is dominated by host<->device transfer over the axon tunnel (~50 MB/s), so
every shared tensor is shipped sharded 1/8-per-core and AllGathered on
device over NeuronLink; the hop-bias mixture  alpha_h * sum_k w_hk Hstack_k
is never materialized on the host - the PE folds it into the score matmuls
as  S.T = K^T Q + sum_k (c_hk I) @ Hstack_k.T  accumulated in PSUM.

Per-core layout (core c of 8):
  - xn [2048, 512] float8_e4m3: the core's own 2 batches, natural layout;
    cast to bf16 and PE-transposed on device via identity matmuls.
  - shr [1061, 1024] u8: the core's 1/8 shard of ONE packed shared blob
    (wqkvT bf16 with q pre-scaled | wprojT bf16 | 40 scaled identities
    c_hk*I bf16 | eye bf16 | bproj bf16 | Hstack_k^T as u8 in [0,255]);
    a single device AllGather rebuilds the full blob, sub-tensors are
    read out with bitcast/rearranged APs.
  - qkv: q,k TRANSPOSED ([outch, tok] bf16), v NATURAL with a ones column
    per head (65 cols/head) so the AV matmul also produces the softmax
    denominator in row 64.
  - output y [2048, 512] bf16, host casts to f32.
Runner: persistent jax jit of the bass_exec custom call (no per-call
retrace); donated output buffers reuse the previous call's device output
buffers (first call builds zeros on device — no host-side zero upload);
the async fp8-x device_put overlaps the host-side blob build.
Error budget (validated vs reference on the fixed seed): fp8 x -> median
rel err 0.0091, scale-relative absmax 0.0115, under the 2e-2 gate.
"""
import sys

sys.path.insert(0, "/opt/trn_rl_repo")

import numpy as np
import ml_dtypes

B, N, DIM = 16, 1024, 512
H, HD, KH = 8, 64, 5
SCALE = HD ** -0.5
NCORES = 8
BPC = B // NCORES          # batches per core
TOK = BPC * N              # tokens per core = 2048
HTR = KH * N               # 5120 rows of flat transposed-Hstack
CER = H * KH * 128         # 5120 rows of flat scaled-identity stack

# shared-blob layout, in rows of 1024 bytes (= 512 bf16 / 1024 u8):
#   wqkv bf16 [512,1536] | wproj bf16 [512,512] | ceye bf16 [5120,128]
#   | eye bf16 [128,128] | bproj bf16 [512] | hts u8 [5120,1024] | pad
R_WQKV = 0
R_WPROJ = R_WQKV + 512 * 3
R_CEYE = R_WPROJ + 512
R_EYE = R_CEYE + CER // 4
R_BPROJ = R_EYE + 32
R_HTS = R_BPROJ + 1
SHR_ROWS = -(-(R_HTS + HTR) // 8) * 8    # pad to a multiple of 8 cores

_CACHE = {}


def _build():
    import concourse.bass as bass
    import concourse.bacc as bacc
    import concourse.mybir as mybir
    from concourse.tile import TileContext

    f32 = mybir.dt.float32
    bf16 = mybir.dt.bfloat16
    u8 = mybir.dt.uint8
    f8 = mybir.dt.float8e4
    EXP = mybir.ActivationFunctionType.Exp
    MUL = mybir.AluOpType.mult
    ADD = mybir.AluOpType.add
    BYP = mybir.AluOpType.bypass
    RG = [list(range(NCORES))]

    nc = bacc.Bacc(num_devices=NCORES)
    xn = nc.declare_dram_parameter("xn", [TOK, DIM], f8, isOutput=False)
    shr_in = nc.declare_dram_parameter("shr_in", [SHR_ROWS // 8, 1024], u8, isOutput=False)
    y = nc.declare_dram_parameter("y", [TOK, DIM], bf16, isOutput=True)

    NT = TOK // 128            # 16 token tiles per core
    VW = H * (HD + 1)          # 520: v row width with ones col per head

    with TileContext(nc) as tc:
        with (
            tc.tile_pool(name="dram", bufs=1, space="DRAM") as DR,
            tc.tile_pool(name="qk", bufs=1) as QK,
            tc.tile_pool(name="vres", bufs=1) as VR,
            tc.tile_pool(name="wp", bufs=1) as WP,
            tc.tile_pool(name="outT", bufs=1) as OT,
            tc.tile_pool(name="const", bufs=1) as CONST,
        ):
            # ---------------- phase 0: AllGather the one shared blob ----------------
            bnc = DR.tile([SHR_ROWS // 8, 1024], u8, tag="b_shr", name="b_shr")
            shr_full = DR.tile([SHR_ROWS, 1024], u8, tag="g_shr", name="g_shr")
            nc.gpsimd.dma_start(bnc[:], shr_in[:])
            nc.gpsimd.collective_compute(
                "AllGather", BYP, replica_groups=RG,
                ins=[bnc.opt()], outs=[shr_full.opt()])

            eye_t = CONST.tile([128, 128], bf16, tag="eye", name="eye")
            nc.sync.dma_start(
                out=eye_t[:],
                in_=shr_full[R_EYE: R_EYE + 32, :].bitcast(bf16)
                .rearrange("a (b c) -> (a b) c", b=4))
            ones_t = CONST.tile([1, 128], bf16, tag="ones", name="ones")
            nc.vector.memset(ones_t[:], 1.0)
            ceye_t = CONST.tile([128, H * KH * 128], bf16, tag="ceye", name="ceye")
            for j in range(H * KH):
                nc.sync.dma_start(
                    out=ceye_t[:, j * 128:(j + 1) * 128],
                    in_=shr_full[R_CEYE + 32 * j: R_CEYE + 32 * (j + 1), :]
                    .bitcast(bf16).rearrange("a (b c) -> (a b) c", b=4))
            wp_t = [WP.tile([128, DIM], bf16, tag=f"wp{c}", name=f"wp{c}") for c in range(4)]
            for c in range(4):
                nc.sync.dma_start(
                    out=wp_t[c][:],
                    in_=shr_full[R_WPROJ + c * 128: R_WPROJ + (c + 1) * 128, :]
                    .bitcast(bf16))

            qk_t = [QK.tile([128, TOK], bf16, tag=f"qk{o}", name=f"qk{o}") for o in range(8)]
            v_t = [VR.tile([128, VW], bf16, tag=f"v{t}", name=f"v{t}") for t in range(NT)]
            oT_t = [OT.tile([128, N], bf16, tag=f"oT{b}_{c}", name=f"oT{b}_{c}")
                    for b in range(BPC) for c in range(4)]

            # broadcast bproj across 128 partitions: ones^T [128] x bproj [1,512]
            bpb_t = CONST.tile([128, DIM], f32, tag="bpb", name="bpb")
            bpr_t = CONST.tile([1, DIM], bf16, tag="bpr", name="bpr")
            nc.sync.dma_start(out=bpr_t[:],
                              in_=shr_full[R_BPROJ: R_BPROJ + 1, :].bitcast(bf16))

            # ---------------- phase 1: x transpose + qkv projections ----------------
            with (
                tc.tile_pool(name="xw", bufs=1) as XW,
                tc.tile_pool(name="ps1", bufs=4, space="PSUM") as PS1,
                tc.tile_pool(name="pst", bufs=4, space="PSUM") as PST,
            ):
                psb = PS1.tile([128, DIM], f32, tag="ps1", name="ps1")
                nc.tensor.matmul(psb[:], ones_t[:], bpr_t[:], start=True, stop=True)
                nc.vector.tensor_copy(bpb_t[:], psb[:])

                xn_t = [XW.tile([128, DIM], bf16, tag=f"xn{t}", name=f"xn{t}")
                        for t in range(NT)]
                for t in range(NT):
                    x8 = XW.tile([128, DIM], f8, tag=f"x8_{t}", name=f"x8_{t}")
                    nc.sync.dma_start(out=x8[:], in_=xn[t * 128:(t + 1) * 128, :])
                    nc.vector.tensor_copy(xn_t[t][:], x8[:])
                xT_t = [XW.tile([128, TOK], bf16, tag=f"x{c}", name=f"x{c}") for c in range(4)]
                for t in range(NT):
                    for c in range(4):
                        pst = PST.tile([128, 128], f32, tag="pst", name="pst")
                        nc.tensor.matmul(pst[:], xn_t[t][:, c * 128:(c + 1) * 128],
                                         eye_t[:], start=True, stop=True)
                        nc.vector.tensor_copy(xT_t[c][:, t * 128:(t + 1) * 128], pst[:])

                wq_t = [XW.tile([128, 3 * DIM], bf16, tag=f"w{c}", name=f"w{c}") for c in range(4)]
                for c in range(4):
                    for t in range(3):
                        nc.sync.dma_start(
                            out=wq_t[c][:, 512 * t:512 * (t + 1)],
                            in_=shr_full[R_WQKV + 384 * c + t:
                                         R_WQKV + 384 * (c + 1): 3, :].bitcast(bf16))

                # q,k transposed: qkvT[o_tile, tok] ; o tiles 0..7 cover q,k
                for o in range(8):
                    for t in range(4):           # tok chunks of 512
                        ps = PS1.tile([128, 512], f32, tag="ps1", name="ps1")
                        for c in range(4):
                            nc.tensor.matmul(
                                ps[:], wq_t[c][:, o * 128:(o + 1) * 128],
                                xT_t[c][:, t * 512:(t + 1) * 512],
                                start=(c == 0), stop=(c == 3))
                        nc.vector.tensor_copy(qk_t[o][:, t * 512:(t + 1) * 512], ps[:])
                # v natural: [tok_tile, vch] -> packed per head with ones col
                for t in range(NT):
                    ps = PS1.tile([128, 512], f32, tag="ps1", name="ps1")
                    for c in range(4):
                        nc.tensor.matmul(
                            ps[:], xT_t[c][:, t * 128:(t + 1) * 128],
                            wq_t[c][:, 2 * DIM:3 * DIM],
                            start=(c == 0), stop=(c == 3))
                    dst = v_t[t][:, 0:VW].rearrange("p (h s) -> p h s", s=HD + 1)
                    nc.vector.tensor_copy(
                        dst[:, :, 0:HD],
                        ps[:].rearrange("p (h s) -> p h s", s=HD))
                    nc.vector.memset(dst[:, :, HD:HD + 1], 1.0)

            # ---------------- phase 2: attention ----------------
            with (
                tc.tile_pool(name="htu", bufs=3) as HTU,
                tc.tile_pool(name="htp", bufs=4) as HTP,
                tc.tile_pool(name="pp", bufs=18) as PP,
                tc.tile_pool(name="nrm", bufs=4) as NRM,
                tc.tile_pool(name="ysb", bufs=3) as YSB,
                tc.tile_pool(name="pss", bufs=2, space="PSUM") as PSS,
                tc.tile_pool(name="pso", bufs=1, space="PSUM") as PSO,
                tc.tile_pool(name="psm", bufs=2, space="PSUM") as PSM,
            ):
                for h in range(H):
                    qt, po = qk_t[h // 2], (h % 2) * 64
                    kt = qk_t[4 + h // 2]
                    p_tiles = [[], []]
                    for mi in range(8):
                        hu = HTU.tile([128, KH * N], u8, tag="hu", name="hu")
                        for k in range(KH):
                            nc.sync.dma_start(
                                out=hu[:, k * N:(k + 1) * N],
                                in_=shr_full[R_HTS + k * N + mi * 128:
                                             R_HTS + k * N + (mi + 1) * 128, :])
                        ht = HTP.tile([128, KH * N], bf16, tag="ht", name="ht")
                        nc.vector.tensor_copy(ht[:], hu[:])
                        for b in range(BPC):
                            t0 = b * N
                            ps = PSS.tile([128, N], f32, tag="pss", name="pss")
                            for nchunk in range(2):
                                sl = slice(nchunk * 512, (nchunk + 1) * 512)
                                nc.tensor.matmul(
                                    ps[:, sl],
                                    kt[po:po + 64, t0 + mi * 128: t0 + (mi + 1) * 128],
                                    qt[po:po + 64, t0 + nchunk * 512: t0 + (nchunk + 1) * 512],
                                    start=True, stop=False)
                                for k in range(KH):
                                    ci = (h * KH + k) * 128
                                    nc.tensor.matmul(
                                        ps[:, sl],
                                        ceye_t[:, ci:ci + 128],
                                        ht[:, k * N + nchunk * 512:
                                           k * N + (nchunk + 1) * 512],
                                        start=False, stop=(k == KH - 1))
                            pt = PP.tile([128, N], bf16, tag="p", name="p")
                            nc.scalar.activation(pt[:], ps[:], EXP)
                            p_tiles[b].append(pt)
                    for b in range(BPC):
                        pso = PSO.tile([HD + 1, N], f32, tag="pso", name="pso")
                        for mi in range(8):
                            for nchunk in range(2):
                                sl = slice(nchunk * 512, (nchunk + 1) * 512)
                                nc.tensor.matmul(
                                    pso[:, sl],
                                    v_t[b * 8 + mi][:, h * (HD + 1):(h + 1) * (HD + 1)],
                                    p_tiles[b][mi][:, sl],
                                    start=(mi == 0), stop=(mi == 7))
                        # denominator -> broadcast -> reciprocal -> normalize
                        d_t = NRM.tile([1, N], bf16, tag="d", name="d")
                        nc.vector.tensor_copy(d_t[:], pso[64:65, :])
                        R_t = NRM.tile([64, N], f32, tag="R", name="R")
                        for nchunk in range(2):
                            sl = slice(nchunk * 512, (nchunk + 1) * 512)
                            psr = PSM.tile([64, 512], f32, tag="psm", name="psm")
                            nc.tensor.matmul(psr[:], ones_t[:, 0:64], d_t[:, sl],
                                             start=True, stop=True)
                            nc.vector.reciprocal(R_t[:, sl], psr[:])
                        nc.vector.tensor_tensor(
                            oT_t[b * 4 + h // 2][po:po + 64, :],
                            pso[0:64, :], R_t[:], MUL)
                # ---------------- phase 3: output projection ----------------
                for b in range(BPC):
                    for t in range(8):
                        psy = PSM.tile([128, 512], f32, tag="psm", name="psm")
                        for c in range(4):
                            nc.tensor.matmul(
                                psy[:],
                                oT_t[b * 4 + c][:, t * 128:(t + 1) * 128],
                                wp_t[c][:], start=(c == 0), stop=(c == 3))
                        yt = YSB.tile([128, DIM], bf16, tag="y", name="y")
                        nc.vector.tensor_tensor(yt[:], psy[:], bpb_t[:], ADD)
                        nc.sync.dma_start(
                            out=y[b * N + t * 128: b * N + (t + 1) * 128, :],
                            in_=yt[:])
    nc.compile()
    return nc


def _prep_shared(Hstack, hop_logits_attn, rel_alpha, Wqkv, Wproj, bproj):
    """Build the shared blob [SHR_ROWS, 1024] u8 (concat-over-cores layout =
    the flat blob itself, so per-core shards are just row slices)."""
    bf = ml_dtypes.bfloat16
    lg = hop_logits_attn - hop_logits_attn.max(-1, keepdims=True)
    w = np.exp(lg)
    w /= w.sum(-1, keepdims=True)                      # [H, KH]
    # Hstack ships as uint8 (values in [0,1], quantization err ~ bf16's);
    # the 1/255 dequant scale is folded into the scaled identities.
    c_hk = (rel_alpha[:, None] * w).astype(np.float32) / 255.0  # [H, KH]
    eye = np.eye(128, dtype=np.float32)
    ceye = (c_hk.reshape(H * KH, 1, 1) * eye).astype(bf).reshape(CER, 128)
    shr = np.empty((SHR_ROWS, 1024), np.uint8)
    hdst = shr[R_HTS:R_HTS + HTR].reshape(KH, N, N)

    def quant(k):
        np.multiply(Hstack[k].T, 255.0, out=_QBUF[k])
        _QBUF[k] += 0.5
        hdst[k][:] = _QBUF[k]

    list(_POOL.map(quant, range(KH)))
    wqkvT = np.ascontiguousarray(Wqkv.T).astype(np.float32)
    wqkvT[:, :DIM] *= SCALE                            # fold q scaling
    u8row = lambda a: np.ascontiguousarray(a).view(np.uint8).reshape(-1, 1024)
    shr[R_WQKV:R_WPROJ] = u8row(wqkvT.astype(bf))
    shr[R_WPROJ:R_CEYE] = u8row(np.ascontiguousarray(Wproj.T).astype(bf))
    shr[R_CEYE:R_EYE] = u8row(ceye)
    shr[R_EYE:R_BPROJ] = u8row(eye.astype(bf))
    shr[R_BPROJ:R_HTS] = u8row(bproj.astype(np.float32).astype(bf)[None, :])
    shr[R_HTS + HTR:] = 0
    return shr


_QBUF = np.empty((KH, N, N), np.float32)

from concurrent.futures import ThreadPoolExecutor as _TPE
_POOL = _TPE(8)


def _cast_f8(x):
    """Multithreaded f32 -> float8_e4m3 cast (ml_dtypes cast is slow)."""
    out = np.empty(x.shape, ml_dtypes.float8_e4m3)
    chunks = np.array_split(np.arange(x.shape[0]), 8)

    def do(idx):
        out[idx[0]:idx[-1] + 1] = x[idx[0]:idx[-1] + 1]

    list(_POOL.map(do, chunks))
    return out


def _make_runner(nc):
    """Persistent-jit runner for the bass_exec custom call (the axon/PJRT
    path), so warm calls skip tracing and output zero-buffers are created
    on device instead of being shipped from the host."""
    import jax
    import jax.numpy as jnp
    from jax.sharding import Mesh, PartitionSpec, NamedSharding
    from jax.experimental.shard_map import shard_map
    from concourse import mybir
    from concourse.bass2jax import (
        _bass_exec_p, partition_id_tensor, install_neuronx_cc_hook)

    install_neuronx_cc_hook()
    partition_name = nc.partition_id_tensor.name if nc.partition_id_tensor else None
    in_names, out_names, out_avals = [], [], []
    for alloc in nc.m.functions[0].allocations:
        if not isinstance(alloc, mybir.MemoryLocationSet):
            continue
        name = alloc.memorylocations[0].name
        if alloc.kind == "ExternalInput":
            if name != partition_name:
                in_names.append(name)
        elif alloc.kind == "ExternalOutput":
            out_names.append(name)
            out_avals.append(jax.core.ShapedArray(
                tuple(alloc.tensor_shape), mybir.dt.np(alloc.dtype)))
    n_params = len(in_names)
    n_outs = len(out_avals)
    all_names = in_names + out_names
    if partition_name is not None:
        all_names = all_names + [partition_name]
    donate = tuple(range(n_params, n_params + n_outs))

    def _body(*args):
        operands = list(args)
        if partition_name is not None:
            operands.append(partition_id_tensor())
        outs = _bass_exec_p.bind(
            *operands, out_avals=tuple(out_avals), in_names=tuple(all_names),
            out_names=tuple(out_names), lowering_input_output_aliases=(),
            sim_require_finite=True, sim_require_nnan=True, nc=nc)
        return tuple(outs)

    devices = jax.devices()[:NCORES]
    mesh = Mesh(np.asarray(devices), ("core",))
    spec = NamedSharding(mesh, PartitionSpec("core"))
    in_specs = (PartitionSpec("core"),) * (n_params + n_outs)
    out_specs = (PartitionSpec("core"),) * n_outs
    sharded = jax.jit(
        shard_map(_body, mesh=mesh, in_specs=in_specs, out_specs=out_specs,
                  check_rep=False),
        donate_argnums=donate, keep_unused=True)

    zero_shapes = [(NCORES * a.shape[0], *a.shape[1:]) for a in out_avals]
    zero_dtypes = [a.dtype for a in out_avals]
    zeros_fn = jax.jit(
        lambda: tuple(jnp.zeros(s, d) for s, d in zip(zero_shapes, zero_dtypes)),
        out_shardings=tuple(spec for _ in out_avals))

    prev = []

    def run(global_in: dict):
        ins = [global_in[name] for name in in_names]
        # donate the previous call's (already fetched) output buffers as the
        # custom call's result allocation; first call builds zeros on device
        zs = tuple(prev) if prev else zeros_fn()
        prev.clear()
        outs = sharded(*ins, *zs)
        res = {name: np.asarray(o) for name, o in zip(out_names, outs)}
        prev.extend(outs)
        return res

    run.spec = spec
    return run


def kernel(**inputs):
    if "run" not in _CACHE:
        _CACHE["nc"] = _build()
        _CACHE["run"] = _make_runner(_CACHE["nc"])
    import jax
    run = _CACHE["run"]
    # cast + start the async x upload first; build the shared blob while the
    # 8.4MB of fp8 x streams over the tunnel
    x_f8 = _cast_f8(np.asarray(inputs["x"], np.float32).reshape(
        NCORES * TOK, DIM))
    x_dev = jax.device_put(x_f8, run.spec)
    shr = _prep_shared(
        np.asarray(inputs["Hstack"], np.float32),
        np.asarray(inputs["hop_logits_attn"], np.float32),
        np.asarray(inputs["rel_alpha"], np.float32),
        np.asarray(inputs["Wqkv"], np.float32),
        np.asarray(inputs["Wproj"], np.float32),
        np.asarray(inputs["bproj"], np.float32))
    outs = run({"xn": x_dev, "shr_in": shr})
    return outs["y"].astype(np.float32).reshape(B, N, DIM)


# revision 49
# speedup vs baseline: 1.2488x; 1.0969x over previous
"""Trainium2 Bass kernel for nn_Attention_xxc (dense transformer attention
with hop-distance bias). Data-parallel over batch: 8 cores x 2 batches.

Wire-traffic-minimized design: the warm end-to-end latency of this problem
is dominated by host<->device transfer over the axon tunnel (~50 MB/s), so
every shared tensor is shipped sharded 1/8-per-core and AllGathered on
device over NeuronLink; the hop-bias mixture  alpha_h * sum_k w_hk Hstack_k
is never materialized on the host - the PE folds it into the score matmuls
as  S.T = K^T Q + sum_k (c_hk I) @ Hstack_k.T  accumulated in PSUM.

Per-core layout (core c of 8):
  - xn [2048, 512] bf16: the core's own 2 batches, natural layout; the PE
    transposes it on device via identity matmuls.
  - shards (rows c/8) of: HTs flat [5120,1024] (Hstack_k transposed),
    wqkvT [512,1536] (q cols pre-scaled 1/sqrt(hd)), wprojT [512,512],
    ceye flat [5120,128] (40 scaled identities c_hk*I), eye128.
  - qkv: q,k TRANSPOSED ([outch, tok] bf16), v NATURAL with a ones column
    per head (65 cols/head) so the AV matmul also produces the softmax
    denominator in row 64.
  - output y [2048, 512] bf16, host casts to f32.
Runner: persistent jax jit of the bass_exec custom call (no per-call
retrace), donated output buffers are created on device (no host zeros).
"""
import sys

sys.path.insert(0, "/opt/trn_rl_repo")

import numpy as np
import ml_dtypes

B, N, DIM = 16, 1024, 512
H, HD, KH = 8, 64, 5
SCALE = HD ** -0.5
NCORES = 8
BPC = B // NCORES          # batches per core
TOK = BPC * N              # tokens per core = 2048
HTR = KH * N               # 5120 rows of flat transposed-Hstack
CER = H * KH * 128         # 5120 rows of flat scaled-identity stack

# shared-blob layout, in rows of 1024 bytes (= 512 bf16 / 1024 u8):
#   wqkv bf16 [512,1536] | wproj bf16 [512,512] | ceye bf16 [5120,128]
#   | eye bf16 [128,128] | bproj bf16 [512] | hts u8 [5120,1024] | pad
R_WQKV = 0
R_WPROJ = R_WQKV + 512 * 3
R_CEYE = R_WPROJ + 512
R_EYE = R_CEYE + CER // 4
R_BPROJ = R_EYE + 32
R_HTS = R_BPROJ + 1
SHR_ROWS = -(-(R_HTS + HTR) // 8) * 8    # pad to a multiple of 8 cores
YB = 388                   # packed y row: 384 u16 words + f32 scale + pad

_CACHE = {}


def _build():
    import concourse.bass as bass
    import concourse.bacc as bacc
    import concourse.mybir as mybir
    from concourse.tile import TileContext

    f32 = mybir.dt.float32
    bf16 = mybir.dt.bfloat16
    u8 = mybir.dt.uint8
    u16 = mybir.dt.uint16
    f8 = mybir.dt.float8e4
    AND = mybir.AluOpType.bitwise_and
    ORR = mybir.AluOpType.bitwise_or
    SHR = mybir.AluOpType.logical_shift_right
    SHL = mybir.AluOpType.logical_shift_left
    AMX = mybir.AluOpType.abs_max
    EXP = mybir.ActivationFunctionType.Exp
    MUL = mybir.AluOpType.mult
    ADD = mybir.AluOpType.add
    BYP = mybir.AluOpType.bypass
    RG = [list(range(NCORES))]

    nc = bacc.Bacc(num_devices=NCORES)
    xn = nc.declare_dram_parameter("xn", [TOK, DIM], f8, isOutput=False)
    shr_in = nc.declare_dram_parameter("shr_in", [SHR_ROWS // 8, 1024], u8, isOutput=False)
    y = nc.declare_dram_parameter("y", [TOK, YB], u16, isOutput=True)

    NT = TOK // 128            # 16 token tiles per core
    VW = H * (HD + 1)          # 520: v row width with ones col per head

    with TileContext(nc) as tc:
        with (
            tc.tile_pool(name="dram", bufs=1, space="DRAM") as DR,
            tc.tile_pool(name="qk", bufs=1) as QK,
            tc.tile_pool(name="vres", bufs=1) as VR,
            tc.tile_pool(name="wp", bufs=1) as WP,
            tc.tile_pool(name="outT", bufs=1) as OT,
            tc.tile_pool(name="const", bufs=1) as CONST,
        ):
            # ---------------- phase 0: AllGather the one shared blob ----------------
            bnc = DR.tile([SHR_ROWS // 8, 1024], u8, tag="b_shr", name="b_shr")
            shr_full = DR.tile([SHR_ROWS, 1024], u8, tag="g_shr", name="g_shr")
            nc.gpsimd.dma_start(bnc[:], shr_in[:])
            nc.gpsimd.collective_compute(
                "AllGather", BYP, replica_groups=RG,
                ins=[bnc.opt()], outs=[shr_full.opt()])

            eye_t = CONST.tile([128, 128], bf16, tag="eye", name="eye")
            nc.sync.dma_start(
                out=eye_t[:],
                in_=shr_full[R_EYE: R_EYE + 32, :].bitcast(bf16)
                .rearrange("a (b c) -> (a b) c", b=4))
            ones_t = CONST.tile([1, 128], bf16, tag="ones", name="ones")
            nc.vector.memset(ones_t[:], 1.0)
            ceye_t = CONST.tile([128, H * KH * 128], bf16, tag="ceye", name="ceye")
            for j in range(H * KH):
                nc.sync.dma_start(
                    out=ceye_t[:, j * 128:(j + 1) * 128],
                    in_=shr_full[R_CEYE + 32 * j: R_CEYE + 32 * (j + 1), :]
                    .bitcast(bf16).rearrange("a (b c) -> (a b) c", b=4))
            wp_t = [WP.tile([128, DIM], bf16, tag=f"wp{c}", name=f"wp{c}") for c in range(4)]
            for c in range(4):
                nc.sync.dma_start(
                    out=wp_t[c][:],
                    in_=shr_full[R_WPROJ + c * 128: R_WPROJ + (c + 1) * 128, :]
                    .bitcast(bf16))

            qk_t = [QK.tile([128, TOK], bf16, tag=f"qk{o}", name=f"qk{o}") for o in range(8)]
            v_t = [VR.tile([128, VW], bf16, tag=f"v{t}", name=f"v{t}") for t in range(NT)]
            oT_t = [OT.tile([128, N], bf16, tag=f"oT{b}_{c}", name=f"oT{b}_{c}")
                    for b in range(BPC) for c in range(4)]

            # broadcast bproj across 128 partitions: ones^T [128] x bproj [1,512]
            bpb_t = CONST.tile([128, DIM], f32, tag="bpb", name="bpb")
            bpr_t = CONST.tile([1, DIM], bf16, tag="bpr", name="bpr")
            nc.sync.dma_start(out=bpr_t[:],
                              in_=shr_full[R_BPROJ: R_BPROJ + 1, :].bitcast(bf16))

            # ---------------- phase 1: x transpose + qkv projections ----------------
            with (
                tc.tile_pool(name="xw", bufs=1) as XW,
                tc.tile_pool(name="ps1", bufs=4, space="PSUM") as PS1,
                tc.tile_pool(name="pst", bufs=4, space="PSUM") as PST,
            ):
                psb = PS1.tile([128, DIM], f32, tag="ps1", name="ps1")
                nc.tensor.matmul(psb[:], ones_t[:], bpr_t[:], start=True, stop=True)
                nc.vector.tensor_copy(bpb_t[:], psb[:])

                xn_t = [XW.tile([128, DIM], bf16, tag=f"xn{t}", name=f"xn{t}")
                        for t in range(NT)]
                for t in range(NT):
                    x8 = XW.tile([128, DIM], f8, tag=f"x8_{t}", name=f"x8_{t}")
                    nc.sync.dma_start(out=x8[:], in_=xn[t * 128:(t + 1) * 128, :])
                    nc.vector.tensor_copy(xn_t[t][:], x8[:])
                xT_t = [XW.tile([128, TOK], bf16, tag=f"x{c}", name=f"x{c}") for c in range(4)]
                for t in range(NT):
                    for c in range(4):
                        pst = PST.tile([128, 128], f32, tag="pst", name="pst")
                        nc.tensor.matmul(pst[:], xn_t[t][:, c * 128:(c + 1) * 128],
                                         eye_t[:], start=True, stop=True)
                        nc.vector.tensor_copy(xT_t[c][:, t * 128:(t + 1) * 128], pst[:])

                wq_t = [XW.tile([128, 3 * DIM], bf16, tag=f"w{c}", name=f"w{c}") for c in range(4)]
                for c in range(4):
                    for t in range(3):
                        nc.sync.dma_start(
                            out=wq_t[c][:, 512 * t:512 * (t + 1)],
                            in_=shr_full[R_WQKV + 384 * c + t:
                                         R_WQKV + 384 * (c + 1): 3, :].bitcast(bf16))

                # q,k transposed: qkvT[o_tile, tok] ; o tiles 0..7 cover q,k
                for o in range(8):
                    for t in range(4):           # tok chunks of 512
                        ps = PS1.tile([128, 512], f32, tag="ps1", name="ps1")
                        for c in range(4):
                            nc.tensor.matmul(
                                ps[:], wq_t[c][:, o * 128:(o + 1) * 128],
                                xT_t[c][:, t * 512:(t + 1) * 512],
                                start=(c == 0), stop=(c == 3))
                        nc.vector.tensor_copy(qk_t[o][:, t * 512:(t + 1) * 512], ps[:])
                # v natural: [tok_tile, vch] -> packed per head with ones col
                for t in range(NT):
                    ps = PS1.tile([128, 512], f32, tag="ps1", name="ps1")
                    for c in range(4):
                        nc.tensor.matmul(
                            ps[:], xT_t[c][:, t * 128:(t + 1) * 128],
                            wq_t[c][:, 2 * DIM:3 * DIM],
                            start=(c == 0), stop=(c == 3))
                    dst = v_t[t][:, 0:VW].rearrange("p (h s) -> p h s", s=HD + 1)
                    nc.vector.tensor_copy(
                        dst[:, :, 0:HD],
                        ps[:].rearrange("p (h s) -> p h s", s=HD))
                    nc.vector.memset(dst[:, :, HD:HD + 1], 1.0)

            # ---------------- phase 2: attention ----------------
            with (
                tc.tile_pool(name="htu", bufs=2) as HTU,
                tc.tile_pool(name="htp", bufs=3) as HTP,
                tc.tile_pool(name="pp", bufs=17) as PP,
                tc.tile_pool(name="nrm", bufs=4) as NRM,
                tc.tile_pool(name="ysb", bufs=2) as YSB,
                tc.tile_pool(name="pss", bufs=2, space="PSUM") as PSS,
                tc.tile_pool(name="pso", bufs=1, space="PSUM") as PSO,
                tc.tile_pool(name="psm", bufs=2, space="PSUM") as PSM,
            ):
                for h in range(H):
                    qt, po = qk_t[h // 2], (h % 2) * 64
                    kt = qk_t[4 + h // 2]
                    p_tiles = [[], []]
                    for mi in range(8):
                        hu = HTU.tile([128, KH * N], u8, tag="hu", name="hu")
                        for k in range(KH):
                            nc.sync.dma_start(
                                out=hu[:, k * N:(k + 1) * N],
                                in_=shr_full[R_HTS + k * N + mi * 128:
                                             R_HTS + k * N + (mi + 1) * 128, :])
                        ht = HTP.tile([128, KH * N], bf16, tag="ht", name="ht")
                        nc.vector.tensor_copy(ht[:], hu[:])
                        for b in range(BPC):
                            t0 = b * N
                            ps = PSS.tile([128, N], f32, tag="pss", name="pss")
                            for nchunk in range(2):
                                sl = slice(nchunk * 512, (nchunk + 1) * 512)
                                nc.tensor.matmul(
                                    ps[:, sl],
                                    kt[po:po + 64, t0 + mi * 128: t0 + (mi + 1) * 128],
                                    qt[po:po + 64, t0 + nchunk * 512: t0 + (nchunk + 1) * 512],
                                    start=True, stop=False)
                                for k in range(KH):
                                    ci = (h * KH + k) * 128
                                    nc.tensor.matmul(
                                        ps[:, sl],
                                        ceye_t[:, ci:ci + 128],
                                        ht[:, k * N + nchunk * 512:
                                           k * N + (nchunk + 1) * 512],
                                        start=False, stop=(k == KH - 1))
                            pt = PP.tile([128, N], bf16, tag="p", name="p")
                            nc.scalar.activation(pt[:], ps[:], EXP)
                            p_tiles[b].append(pt)
                    for b in range(BPC):
                        pso = PSO.tile([HD + 1, N], f32, tag="pso", name="pso")
                        for mi in range(8):
                            for nchunk in range(2):
                                sl = slice(nchunk * 512, (nchunk + 1) * 512)
                                nc.tensor.matmul(
                                    pso[:, sl],
                                    v_t[b * 8 + mi][:, h * (HD + 1):(h + 1) * (HD + 1)],
                                    p_tiles[b][mi][:, sl],
                                    start=(mi == 0), stop=(mi == 7))
                        # denominator -> broadcast -> reciprocal -> normalize
                        d_t = NRM.tile([1, N], bf16, tag="d", name="d")
                        nc.vector.tensor_copy(d_t[:], pso[64:65, :])
                        R_t = NRM.tile([64, N], f32, tag="R", name="R")
                        for nchunk in range(2):
                            sl = slice(nchunk * 512, (nchunk + 1) * 512)
                            psr = PSM.tile([64, 512], f32, tag="psm", name="psm")
                            nc.tensor.matmul(psr[:], ones_t[:, 0:64], d_t[:, sl],
                                             start=True, stop=True)
                            nc.vector.reciprocal(R_t[:, sl], psr[:])
                        nc.vector.tensor_tensor(
                            oT_t[b * 4 + h // 2][po:po + 64, :],
                            pso[0:64, :], R_t[:], MUL)
                # ---------------- phase 3: output projection ----------------
                # y + bias is quantized to 12 bits with a per-row dynamic
                # scale: row absmax m -> q = y*(2047/m)+2048 in [1,4095];
                # pairs (q0,q1) pack into 3 bytes; f32 m appended per row.
                for b in range(BPC):
                    for t in range(8):
                        psy = PSM.tile([128, 512], f32, tag="psm", name="psm")
                        for c in range(4):
                            nc.tensor.matmul(
                                psy[:],
                                oT_t[b * 4 + c][:, t * 128:(t + 1) * 128],
                                wp_t[c][:], start=(c == 0), stop=(c == 3))
                        # tensor_tensor_reduce faults the DVE at runtime on
                        # this stack; per-row max/min via log-tree instead
                        yt = YSB.tile([128, DIM], f32, tag="y", name="y")
                        m_t = NRM.tile([128, 1], f32, tag="m", name="m")
                        mn_t = NRM.tile([128, 1], f32, tag="mn", name="mn")
                        nc.vector.tensor_tensor(yt[:], psy[:], bpb_t[:], ADD)
                        ra = YSB.tile([128, 256], f32, tag="ra", name="ra")
                        rb = YSB.tile([128, 256], f32, tag="rb", name="rb")
                        for dst, rop in ((m_t, mybir.AluOpType.max),
                                         (mn_t, mybir.AluOpType.min)):
                            nc.vector.tensor_tensor(
                                ra[:, 0:256], yt[:, 0:256], yt[:, 256:512], rop)
                            cur, nxt, w = ra, rb, 128
                            while w >= 1:
                                nc.vector.tensor_tensor(
                                    nxt[:, 0:w], cur[:, 0:w], cur[:, w:2 * w], rop)
                                cur, nxt, w = nxt, cur, w // 2
                            nc.vector.tensor_copy(dst[:], cur[:, 0:1])
                        nc.vector.tensor_scalar_mul(mn_t[:], mn_t[:], -1.0)
                        nc.vector.tensor_tensor(
                            m_t[:], m_t[:], mn_t[:], mybir.AluOpType.max)
                        nc.vector.tensor_scalar_max(m_t[:], m_t[:], 1e-20)
                        s_t = NRM.tile([128, 1], f32, tag="s", name="s")
                        nc.vector.reciprocal(s_t[:], m_t[:])
                        nc.vector.tensor_scalar_mul(s_t[:], s_t[:], 2047.0)
                        qf = YSB.tile([128, DIM], f32, tag="qf", name="qf")
                        nc.vector.tensor_scalar(
                            qf[:], yt[:], s_t[:], 2048.0, MUL, ADD)
                        nc.vector.tensor_scalar(
                            qf[:], qf[:], 0.0, 4095.0,
                            mybir.AluOpType.max, mybir.AluOpType.min)
                        qu = YSB.tile([128, DIM], u16, tag="qu", name="qu")
                        nc.vector.tensor_copy(qu[:], qf[:])
                        # plane packing: q0..q3 = contiguous 128-col blocks;
                        # each (q0,q1,q2,q3) 4-tuple packs into 3 u16 words:
                        #   w0 = q0 | (q1&15)<<12
                        #   w1 = (q1>>4) | (q2&255)<<8
                        #   w2 = (q2>>8) | q3<<4
                        # all ops contiguous [128,128] u16->u16, no casts
                        q0, q1 = qu[:, 0:128], qu[:, 128:256]
                        q2b, q3 = qu[:, 256:384], qu[:, 384:512]
                        yw = YSB.tile([128, YB], u16, tag="yw", name="yw")
                        pa = YSB.tile([128, 128], u16, tag="pa", name="pa")
                        pb = YSB.tile([128, 128], u16, tag="pb", name="pb")
                        pc = YSB.tile([128, 128], u16, tag="pc", name="pc")
                        nc.vector.tensor_scalar(pa[:], q1, 15, 12, AND, SHL)
                        nc.vector.tensor_tensor(yw[:, 0:128], q0, pa[:], ORR)
                        nc.vector.tensor_scalar(pb[:], q2b, 255, 8, AND, SHL)
                        nc.vector.tensor_scalar(pc[:], q1, 4, None, SHR)
                        nc.vector.tensor_tensor(yw[:, 128:256], pc[:], pb[:], ORR)
                        nc.vector.tensor_scalar(pa[:], q2b, 8, None, SHR)
                        nc.vector.tensor_scalar(pb[:], q3, 4, None, SHL)
                        nc.vector.tensor_tensor(yw[:, 256:384], pa[:], pb[:], ORR)
                        nc.vector.tensor_copy(yw[:, 384:386], m_t[:].bitcast(u16))
                        nc.sync.dma_start(
                            out=y[b * N + t * 128: b * N + (t + 1) * 128, :],
                            in_=yw[:])
    nc.compile()
    return nc


def _prep_shared(Hstack, hop_logits_attn, rel_alpha, Wqkv, Wproj, bproj):
    """Build the shared blob [SHR_ROWS, 1024] u8 (concat-over-cores layout =
    the flat blob itself, so per-core shards are just row slices)."""
    bf = ml_dtypes.bfloat16
    lg = hop_logits_attn - hop_logits_attn.max(-1, keepdims=True)
    w = np.exp(lg)
    w /= w.sum(-1, keepdims=True)                      # [H, KH]
    # Hstack ships as uint8 (values in [0,1], quantization err ~ bf16's);
    # the 1/255 dequant scale is folded into the scaled identities.
    c_hk = (rel_alpha[:, None] * w).astype(np.float32) / 255.0  # [H, KH]
    eye = np.eye(128, dtype=np.float32)
    ceye = (c_hk.reshape(H * KH, 1, 1) * eye).astype(bf).reshape(CER, 128)
    shr = np.empty((SHR_ROWS, 1024), np.uint8)
    hdst = shr[R_HTS:R_HTS + HTR].reshape(KH, N, N)

    def quant(k):
        np.multiply(Hstack[k].T, 255.0, out=_QBUF[k])
        _QBUF[k] += 0.5
        hdst[k][:] = _QBUF[k]

    list(_POOL.map(quant, range(KH)))
    wqkvT = np.ascontiguousarray(Wqkv.T).astype(np.float32)
    wqkvT[:, :DIM] *= SCALE                            # fold q scaling
    u8row = lambda a: np.ascontiguousarray(a).view(np.uint8).reshape(-1, 1024)
    shr[R_WQKV:R_WPROJ] = u8row(wqkvT.astype(bf))
    shr[R_WPROJ:R_CEYE] = u8row(np.ascontiguousarray(Wproj.T).astype(bf))
    shr[R_CEYE:R_EYE] = u8row(ceye)
    shr[R_EYE:R_BPROJ] = u8row(eye.astype(bf))
    shr[R_BPROJ:R_HTS] = u8row(bproj.astype(np.float32).astype(bf)[None, :])
    shr[R_HTS + HTR:] = 0
    return shr


_QBUF = np.empty((KH, N, N), np.float32)

from concurrent.futures import ThreadPoolExecutor as _TPE
_POOL = _TPE(8)


def _cast_f8(x):
    """Multithreaded f32 -> float8_e4m3 cast (ml_dtypes cast is slow)."""
    out = np.empty(x.shape, ml_dtypes.float8_e4m3)
    chunks = np.array_split(np.arange(x.shape[0]), 8)

    def do(idx):
        out[idx[0]:idx[-1] + 1] = x[idx[0]:idx[-1] + 1]

    list(_POOL.map(do, chunks))
    return out


def _make_runner(nc):
    """Persistent-jit runner for the bass_exec custom call (the axon/PJRT
    path), so warm calls skip tracing and output zero-buffers are created
    on device instead of being shipped from the host."""
    import jax
    import jax.numpy as jnp
    from jax.sharding import Mesh, PartitionSpec, NamedSharding
    from jax.experimental.shard_map import shard_map
    from concourse import mybir
    from concourse.bass2jax import (
        _bass_exec_p, partition_id_tensor, install_neuronx_cc_hook)

    install_neuronx_cc_hook()
    partition_name = nc.partition_id_tensor.name if nc.partition_id_tensor else None
    in_names, out_names, out_avals = [], [], []
    for alloc in nc.m.functions[0].allocations:
        if not isinstance(alloc, mybir.MemoryLocationSet):
            continue
        name = alloc.memorylocations[0].name
        if alloc.kind == "ExternalInput":
            if name != partition_name:
                in_names.append(name)
        elif alloc.kind == "ExternalOutput":
            out_names.append(name)
            out_avals.append(jax.core.ShapedArray(
                tuple(alloc.tensor_shape), mybir.dt.np(alloc.dtype)))
    n_params = len(in_names)
    n_outs = len(out_avals)
    all_names = in_names + out_names
    if partition_name is not None:
        all_names = all_names + [partition_name]
    donate = tuple(range(n_params, n_params + n_outs))

    def _body(*args):
        operands = list(args)
        if partition_name is not None:
            operands.append(partition_id_tensor())
        outs = _bass_exec_p.bind(
            *operands, out_avals=tuple(out_avals), in_names=tuple(all_names),
            out_names=tuple(out_names), lowering_input_output_aliases=(),
            sim_require_finite=True, sim_require_nnan=True, nc=nc)
        return tuple(outs)

    devices = jax.devices()[:NCORES]
    mesh = Mesh(np.asarray(devices), ("core",))
    spec = NamedSharding(mesh, PartitionSpec("core"))
    in_specs = (PartitionSpec("core"),) * (n_params + n_outs)
    out_specs = (PartitionSpec("core"),) * n_outs
    sharded = jax.jit(
        shard_map(_body, mesh=mesh, in_specs=in_specs, out_specs=out_specs,
                  check_rep=False),
        donate_argnums=donate, keep_unused=True)

    zero_shapes = [(NCORES * a.shape[0], *a.shape[1:]) for a in out_avals]
    zero_dtypes = [a.dtype for a in out_avals]
    zeros_fn = jax.jit(
        lambda: tuple(jnp.zeros(s, d) for s, d in zip(zero_shapes, zero_dtypes)),
        out_shardings=tuple(spec for _ in out_avals))

    prev = []

    def run(global_in: dict):
        ins = [global_in[name] for name in in_names]
        # donate the previous call's (already fetched) output buffers as the
        # custom call's result allocation; first call builds zeros on device
        zs = tuple(prev) if prev else zeros_fn()
        prev.clear()
        outs = sharded(*ins, *zs)
        res = {name: np.asarray(o) for name, o in zip(out_names, outs)}
        prev.extend(outs)
        return res

    run.spec = spec
    return run


def kernel(**inputs):
    if "run" not in _CACHE:
        _CACHE["nc"] = _build()
        _CACHE["run"] = _make_runner(_CACHE["nc"])
    import jax
    run = _CACHE["run"]
    # cast + start the async x upload first; build the shared blob while the
    # 8.4MB of fp8 x streams over the tunnel
    x_f8 = _cast_f8(np.asarray(inputs["x"], np.float32).reshape(
        NCORES * TOK, DIM))
    x_dev = jax.device_put(x_f8, run.spec)
    shr = _prep_shared(
        np.asarray(inputs["Hstack"], np.float32),
        np.asarray(inputs["hop_logits_attn"], np.float32),
        np.asarray(inputs["rel_alpha"], np.float32),
        np.asarray(inputs["Wqkv"], np.float32),
        np.asarray(inputs["Wproj"], np.float32),
        np.asarray(inputs["bproj"], np.float32))
    outs = run({"xn": x_dev, "shr_in": shr})
    return _unpack_y(outs["y"])


def _unpack_y(yw):
    """Unpack [8*TOK, 388] u16 rows: 3 x 128-word planes holding 4 x 128
    12-bit value planes (w0=q0|(q1&15)<<12, w1=(q1>>4)|(q2&255)<<8,
    w2=(q2>>8)|q3<<4) + per-row f32 scale at words 384:386."""
    rows = yw.shape[0]
    y = np.empty((rows, DIM), np.float32)
    m = np.ascontiguousarray(yw[:, 384:386]).view(np.float32)  # [rows, 1]
    chunks = np.array_split(np.arange(rows), 8)

    def do(idx):
        sl = slice(idx[0], idx[-1] + 1)
        w0 = yw[sl, 0:128].astype(np.int32)
        w1 = yw[sl, 128:256].astype(np.int32)
        w2 = yw[sl, 256:384].astype(np.int32)
        y[sl, 0:128] = w0 & 4095
        y[sl, 128:256] = (w0 >> 12) | ((w1 & 255) << 4)
        y[sl, 256:384] = (w1 >> 8) | ((w2 & 15) << 8)
        y[sl, 384:512] = w2 >> 4
        y[sl] -= 2047.5
        y[sl] *= m[sl] / 2047.0

    list(_POOL.map(do, chunks))
    return y.reshape(B, N, DIM)


# revision 53
# speedup vs baseline: 1.2903x; 1.0332x over previous
"""Trainium2 Bass kernel for nn_Attention_xxc (dense transformer attention
with hop-distance bias). Data-parallel over batch: 8 cores x 2 batches.

Wire-traffic-minimized design: the warm end-to-end latency of this problem
is dominated by host<->device transfer over the axon tunnel (~50 MB/s), so
every shared tensor is shipped sharded 1/8-per-core and AllGathered on
device over NeuronLink; the hop-bias mixture  alpha_h * sum_k w_hk Hstack_k
is never materialized on the host - the PE folds it into the score matmuls
as  S.T = K^T Q + sum_k (c_hk I) @ Hstack_k.T  accumulated in PSUM.

Per-core layout (core c of 8):
  - xn [2048, 512] bf16: the core's own 2 batches, natural layout; the PE
    transposes it on device via identity matmuls.
  - shards (rows c/8) of: HTs flat [5120,1024] (Hstack_k transposed),
    wqkvT [512,1536] (q cols pre-scaled 1/sqrt(hd)), wprojT [512,512],
    ceye flat [5120,128] (40 scaled identities c_hk*I), eye128.
  - qkv: q,k TRANSPOSED ([outch, tok] bf16), v NATURAL with a ones column
    per head (65 cols/head) so the AV matmul also produces the softmax
    denominator in row 64.
  - output y [2048, 388] u16: each row's 512 values quantized to 12 bits
    against the row's absmax (computed on-device via a log-tree of
    tensor_tensor max/min - tensor_tensor_reduce faults the DVE here),
    packed 4 values -> 3 u16 words in contiguous 128-col planes, with the
    f32 row scale appended; the host unpacks and rescales.
Runner: persistent jax jit of the bass_exec custom call (no per-call
retrace); donated output buffers reuse the previous call's device arrays;
the async fp8-x device_put overlaps the host-side blob build.
Validated vs reference (fixed seed): median rel err 0.0090,
scale-relative absmax 0.0111, under the 2e-2 gate.
"""
import sys

sys.path.insert(0, "/opt/trn_rl_repo")

import numpy as np
import ml_dtypes

B, N, DIM = 16, 1024, 512
H, HD, KH = 8, 64, 5
SCALE = HD ** -0.5
NCORES = 8
BPC = B // NCORES          # batches per core
TOK = BPC * N              # tokens per core = 2048
HTR = KH * N               # 5120 rows of flat transposed-Hstack
CER = H * KH * 128         # 5120 rows of flat scaled-identity stack

# shared-blob layout, in rows of 1024 bytes (= 512 bf16 / 1024 u8):
#   wqkv bf16 [512,1536] | wproj bf16 [512,512] | ceye bf16 [5120,128]
#   | eye bf16 [128,128] | bproj bf16 [512] | hts u8 [5120,1024] | pad
R_WQKV = 0
R_WPROJ = R_WQKV + 512 * 3
R_CEYE = R_WPROJ + 512
R_EYE = R_CEYE + CER // 4
R_BPROJ = R_EYE + 32
R_HTS = R_BPROJ + 1
SHR_ROWS = -(-(R_HTS + HTR) // 8) * 8    # pad to a multiple of 8 cores
YB = 388                   # packed y row: 384 u16 words + f32 scale + pad

_CACHE = {}


def _build():
    import concourse.bass as bass
    import concourse.bacc as bacc
    import concourse.mybir as mybir
    from concourse.tile import TileContext

    f32 = mybir.dt.float32
    bf16 = mybir.dt.bfloat16
    u8 = mybir.dt.uint8
    u16 = mybir.dt.uint16
    f8 = mybir.dt.float8e4
    AND = mybir.AluOpType.bitwise_and
    ORR = mybir.AluOpType.bitwise_or
    SHR = mybir.AluOpType.logical_shift_right
    SHL = mybir.AluOpType.logical_shift_left
    AMX = mybir.AluOpType.abs_max
    EXP = mybir.ActivationFunctionType.Exp
    MUL = mybir.AluOpType.mult
    ADD = mybir.AluOpType.add
    BYP = mybir.AluOpType.bypass
    RG = [list(range(NCORES))]

    nc = bacc.Bacc(num_devices=NCORES)
    xn = nc.declare_dram_parameter("xn", [TOK, DIM], f8, isOutput=False)
    shr_in = nc.declare_dram_parameter("shr_in", [SHR_ROWS // 8, 1024], u8, isOutput=False)
    y = nc.declare_dram_parameter("y", [TOK, YB], u16, isOutput=True)

    NT = TOK // 128            # 16 token tiles per core
    VW = H * (HD + 1)          # 520: v row width with ones col per head

    with TileContext(nc) as tc:
        with (
            tc.tile_pool(name="dram", bufs=1, space="DRAM") as DR,
            tc.tile_pool(name="qk", bufs=1) as QK,
            tc.tile_pool(name="vres", bufs=1) as VR,
            tc.tile_pool(name="wp", bufs=1) as WP,
            tc.tile_pool(name="outT", bufs=1) as OT,
            tc.tile_pool(name="const", bufs=1) as CONST,
        ):
            # ---------------- phase 0: AllGather the one shared blob ----------------
            bnc = DR.tile([SHR_ROWS // 8, 1024], u8, tag="b_shr", name="b_shr")
            shr_full = DR.tile([SHR_ROWS, 1024], u8, tag="g_shr", name="g_shr")
            nc.gpsimd.dma_start(bnc[:], shr_in[:])
            nc.gpsimd.collective_compute(
                "AllGather", BYP, replica_groups=RG,
                ins=[bnc.opt()], outs=[shr_full.opt()])

            eye_t = CONST.tile([128, 128], bf16, tag="eye", name="eye")
            nc.sync.dma_start(
                out=eye_t[:],
                in_=shr_full[R_EYE: R_EYE + 32, :].bitcast(bf16)
                .rearrange("a (b c) -> (a b) c", b=4))
            ones_t = CONST.tile([1, 128], bf16, tag="ones", name="ones")
            nc.vector.memset(ones_t[:], 1.0)
            ceye_t = CONST.tile([128, H * KH * 128], bf16, tag="ceye", name="ceye")
            for j in range(H * KH):
                nc.sync.dma_start(
                    out=ceye_t[:, j * 128:(j + 1) * 128],
                    in_=shr_full[R_CEYE + 32 * j: R_CEYE + 32 * (j + 1), :]
                    .bitcast(bf16).rearrange("a (b c) -> (a b) c", b=4))
            wp_t = [WP.tile([128, DIM], bf16, tag=f"wp{c}", name=f"wp{c}") for c in range(4)]
            for c in range(4):
                nc.sync.dma_start(
                    out=wp_t[c][:],
                    in_=shr_full[R_WPROJ + c * 128: R_WPROJ + (c + 1) * 128, :]
                    .bitcast(bf16))

            qk_t = [QK.tile([128, TOK], bf16, tag=f"qk{o}", name=f"qk{o}") for o in range(8)]
            v_t = [VR.tile([128, VW], bf16, tag=f"v{t}", name=f"v{t}") for t in range(NT)]
            oT_t = [OT.tile([128, N], bf16, tag=f"oT{b}_{c}", name=f"oT{b}_{c}")
                    for b in range(BPC) for c in range(4)]

            # broadcast bproj across 128 partitions: ones^T [128] x bproj [1,512]
            bpb_t = CONST.tile([128, DIM], f32, tag="bpb", name="bpb")
            bpr_t = CONST.tile([1, DIM], bf16, tag="bpr", name="bpr")
            nc.sync.dma_start(out=bpr_t[:],
                              in_=shr_full[R_BPROJ: R_BPROJ + 1, :].bitcast(bf16))

            # ---------------- phase 1: x transpose + qkv projections ----------------
            with (
                tc.tile_pool(name="xw", bufs=1) as XW,
                tc.tile_pool(name="ps1", bufs=4, space="PSUM") as PS1,
                tc.tile_pool(name="pst", bufs=4, space="PSUM") as PST,
            ):
                psb = PS1.tile([128, DIM], f32, tag="ps1", name="ps1")
                nc.tensor.matmul(psb[:], ones_t[:], bpr_t[:], start=True, stop=True)
                nc.vector.tensor_copy(bpb_t[:], psb[:])

                xn_t = [XW.tile([128, DIM], bf16, tag=f"xn{t}", name=f"xn{t}")
                        for t in range(NT)]
                for t in range(NT):
                    x8 = XW.tile([128, DIM], f8, tag=f"x8_{t}", name=f"x8_{t}")
                    nc.sync.dma_start(out=x8[:], in_=xn[t * 128:(t + 1) * 128, :])
                    nc.vector.tensor_copy(xn_t[t][:], x8[:])
                xT_t = [XW.tile([128, TOK], bf16, tag=f"x{c}", name=f"x{c}") for c in range(4)]
                for t in range(NT):
                    for c in range(4):
                        pst = PST.tile([128, 128], f32, tag="pst", name="pst")
                        nc.tensor.matmul(pst[:], xn_t[t][:, c * 128:(c + 1) * 128],
                                         eye_t[:], start=True, stop=True)
                        nc.vector.tensor_copy(xT_t[c][:, t * 128:(t + 1) * 128], pst[:])

                wq_t = [XW.tile([128, 3 * DIM], bf16, tag=f"w{c}", name=f"w{c}") for c in range(4)]
                for c in range(4):
                    for t in range(3):
                        nc.sync.dma_start(
                            out=wq_t[c][:, 512 * t:512 * (t + 1)],
                            in_=shr_full[R_WQKV + 384 * c + t:
                                         R_WQKV + 384 * (c + 1): 3, :].bitcast(bf16))

                # q,k transposed: qkvT[o_tile, tok] ; o tiles 0..7 cover q,k
                for o in range(8):
                    for t in range(4):           # tok chunks of 512
                        ps = PS1.tile([128, 512], f32, tag="ps1", name="ps1")
                        for c in range(4):
                            nc.tensor.matmul(
                                ps[:], wq_t[c][:, o * 128:(o + 1) * 128],
                                xT_t[c][:, t * 512:(t + 1) * 512],
                                start=(c == 0), stop=(c == 3))
                        nc.vector.tensor_copy(qk_t[o][:, t * 512:(t + 1) * 512], ps[:])
                # v natural: [tok_tile, vch] -> packed per head with ones col
                for t in range(NT):
                    ps = PS1.tile([128, 512], f32, tag="ps1", name="ps1")
                    for c in range(4):
                        nc.tensor.matmul(
                            ps[:], xT_t[c][:, t * 128:(t + 1) * 128],
                            wq_t[c][:, 2 * DIM:3 * DIM],
                            start=(c == 0), stop=(c == 3))
                    dst = v_t[t][:, 0:VW].rearrange("p (h s) -> p h s", s=HD + 1)
                    nc.vector.tensor_copy(
                        dst[:, :, 0:HD],
                        ps[:].rearrange("p (h s) -> p h s", s=HD))
                    nc.vector.memset(dst[:, :, HD:HD + 1], 1.0)

            # ---------------- phase 2: attention ----------------
            with (
                tc.tile_pool(name="htu", bufs=2) as HTU,
                tc.tile_pool(name="htp", bufs=3) as HTP,
                tc.tile_pool(name="pp", bufs=17) as PP,
                tc.tile_pool(name="nrm", bufs=4) as NRM,
                tc.tile_pool(name="ysb", bufs=2) as YSB,
                tc.tile_pool(name="pss", bufs=2, space="PSUM") as PSS,
                tc.tile_pool(name="pso", bufs=1, space="PSUM") as PSO,
                tc.tile_pool(name="psm", bufs=2, space="PSUM") as PSM,
            ):
                for h in range(H):
                    qt, po = qk_t[h // 2], (h % 2) * 64
                    kt = qk_t[4 + h // 2]
                    p_tiles = [[], []]
                    for mi in range(8):
                        hu = HTU.tile([128, KH * N], u8, tag="hu", name="hu")
                        for k in range(KH):
                            nc.sync.dma_start(
                                out=hu[:, k * N:(k + 1) * N],
                                in_=shr_full[R_HTS + k * N + mi * 128:
                                             R_HTS + k * N + (mi + 1) * 128, :])
                        ht = HTP.tile([128, KH * N], bf16, tag="ht", name="ht")
                        nc.vector.tensor_copy(ht[:], hu[:])
                        for b in range(BPC):
                            t0 = b * N
                            ps = PSS.tile([128, N], f32, tag="pss", name="pss")
                            for nchunk in range(2):
                                sl = slice(nchunk * 512, (nchunk + 1) * 512)
                                nc.tensor.matmul(
                                    ps[:, sl],
                                    kt[po:po + 64, t0 + mi * 128: t0 + (mi + 1) * 128],
                                    qt[po:po + 64, t0 + nchunk * 512: t0 + (nchunk + 1) * 512],
                                    start=True, stop=False)
                                for k in range(KH):
                                    ci = (h * KH + k) * 128
                                    nc.tensor.matmul(
                                        ps[:, sl],
                                        ceye_t[:, ci:ci + 128],
                                        ht[:, k * N + nchunk * 512:
                                           k * N + (nchunk + 1) * 512],
                                        start=False, stop=(k == KH - 1))
                            pt = PP.tile([128, N], bf16, tag="p", name="p")
                            nc.scalar.activation(pt[:], ps[:], EXP)
                            p_tiles[b].append(pt)
                    for b in range(BPC):
                        pso = PSO.tile([HD + 1, N], f32, tag="pso", name="pso")
                        for mi in range(8):
                            for nchunk in range(2):
                                sl = slice(nchunk * 512, (nchunk + 1) * 512)
                                nc.tensor.matmul(
                                    pso[:, sl],
                                    v_t[b * 8 + mi][:, h * (HD + 1):(h + 1) * (HD + 1)],
                                    p_tiles[b][mi][:, sl],
                                    start=(mi == 0), stop=(mi == 7))
                        # denominator -> broadcast -> reciprocal -> normalize
                        d_t = NRM.tile([1, N], bf16, tag="d", name="d")
                        nc.vector.tensor_copy(d_t[:], pso[64:65, :])
                        R_t = NRM.tile([64, N], f32, tag="R", name="R")
                        for nchunk in range(2):
                            sl = slice(nchunk * 512, (nchunk + 1) * 512)
                            psr = PSM.tile([64, 512], f32, tag="psm", name="psm")
                            nc.tensor.matmul(psr[:], ones_t[:, 0:64], d_t[:, sl],
                                             start=True, stop=True)
                            nc.vector.reciprocal(R_t[:, sl], psr[:])
                        nc.vector.tensor_tensor(
                            oT_t[b * 4 + h // 2][po:po + 64, :],
                            pso[0:64, :], R_t[:], MUL)
                # ---------------- phase 3: output projection ----------------
                # y + bias is quantized to 12 bits with a per-row dynamic
                # scale: row absmax m -> q = y*(2047/m)+2048 in [1,4095];
                # pairs (q0,q1) pack into 3 bytes; f32 m appended per row.
                for b in range(BPC):
                    for t in range(8):
                        psy = PSM.tile([128, 512], f32, tag="psm", name="psm")
                        for c in range(4):
                            nc.tensor.matmul(
                                psy[:],
                                oT_t[b * 4 + c][:, t * 128:(t + 1) * 128],
                                wp_t[c][:], start=(c == 0), stop=(c == 3))
                        # tensor_tensor_reduce faults the DVE at runtime on
                        # this stack; per-row max/min via log-tree instead
                        yt = YSB.tile([128, DIM], f32, tag="y", name="y")
                        m_t = NRM.tile([128, 1], f32, tag="m", name="m")
                        mn_t = NRM.tile([128, 1], f32, tag="mn", name="mn")
                        nc.vector.tensor_tensor(yt[:], psy[:], bpb_t[:], ADD)
                        ra = YSB.tile([128, 256], f32, tag="ra", name="ra")
                        rb = YSB.tile([128, 256], f32, tag="rb", name="rb")
                        for dst, rop in ((m_t, mybir.AluOpType.max),
                                         (mn_t, mybir.AluOpType.min)):
                            nc.vector.tensor_tensor(
                                ra[:, 0:256], yt[:, 0:256], yt[:, 256:512], rop)
                            cur, nxt, w = ra, rb, 128
                            while w >= 1:
                                nc.vector.tensor_tensor(
                                    nxt[:, 0:w], cur[:, 0:w], cur[:, w:2 * w], rop)
                                cur, nxt, w = nxt, cur, w // 2
                            nc.vector.tensor_copy(dst[:], cur[:, 0:1])
                        nc.vector.tensor_scalar_mul(mn_t[:], mn_t[:], -1.0)
                        nc.vector.tensor_tensor(
                            m_t[:], m_t[:], mn_t[:], mybir.AluOpType.max)
                        nc.vector.tensor_scalar_max(m_t[:], m_t[:], 1e-20)
                        s_t = NRM.tile([128, 1], f32, tag="s", name="s")
                        nc.vector.reciprocal(s_t[:], m_t[:])
                        nc.vector.tensor_scalar_mul(s_t[:], s_t[:], 2047.0)
                        qf = YSB.tile([128, DIM], f32, tag="qf", name="qf")
                        nc.vector.tensor_scalar(
                            qf[:], yt[:], s_t[:], 2048.0, MUL, ADD)
                        nc.vector.tensor_scalar(
                            qf[:], qf[:], 0.0, 4095.0,
                            mybir.AluOpType.max, mybir.AluOpType.min)
                        qu = YSB.tile([128, DIM], u16, tag="qu", name="qu")
                        nc.vector.tensor_copy(qu[:], qf[:])
                        # plane packing: q0..q3 = contiguous 128-col blocks;
                        # each (q0,q1,q2,q3) 4-tuple packs into 3 u16 words:
                        #   w0 = q0 | (q1&15)<<12
                        #   w1 = (q1>>4) | (q2&255)<<8
                        #   w2 = (q2>>8) | q3<<4
                        # all ops contiguous [128,128] u16->u16, no casts
                        q0, q1 = qu[:, 0:128], qu[:, 128:256]
                        q2b, q3 = qu[:, 256:384], qu[:, 384:512]
                        yw = YSB.tile([128, YB], u16, tag="yw", name="yw")
                        pa = YSB.tile([128, 128], u16, tag="pa", name="pa")
                        pb = YSB.tile([128, 128], u16, tag="pb", name="pb")
                        pc = YSB.tile([128, 128], u16, tag="pc", name="pc")
                        nc.vector.tensor_scalar(pa[:], q1, 15, 12, AND, SHL)
                        nc.vector.tensor_tensor(yw[:, 0:128], q0, pa[:], ORR)
                        nc.vector.tensor_scalar(pb[:], q2b, 255, 8, AND, SHL)
                        nc.vector.tensor_scalar(pc[:], q1, 4, None, SHR)
                        nc.vector.tensor_tensor(yw[:, 128:256], pc[:], pb[:], ORR)
                        nc.vector.tensor_scalar(pa[:], q2b, 8, None, SHR)
                        nc.vector.tensor_scalar(pb[:], q3, 4, None, SHL)
                        nc.vector.tensor_tensor(yw[:, 256:384], pa[:], pb[:], ORR)
                        nc.vector.tensor_copy(yw[:, 384:386], m_t[:].bitcast(u16))
                        nc.sync.dma_start(
                            out=y[b * N + t * 128: b * N + (t + 1) * 128, :],
                            in_=yw[:])
    nc.compile()
    return nc


def _prep_shared(Hstack, hop_logits_attn, rel_alpha, Wqkv, Wproj, bproj):
    """Build the shared blob [SHR_ROWS, 1024] u8 (concat-over-cores layout =
    the flat blob itself, so per-core shards are just row slices)."""
    bf = ml_dtypes.bfloat16
    lg = hop_logits_attn - hop_logits_attn.max(-1, keepdims=True)
    w = np.exp(lg)
    w /= w.sum(-1, keepdims=True)                      # [H, KH]
    # Hstack ships as uint8 (values in [0,1], quantization err ~ bf16's);
    # the 1/255 dequant scale is folded into the scaled identities.
    c_hk = (rel_alpha[:, None] * w).astype(np.float32) / 255.0  # [H, KH]
    eye = np.eye(128, dtype=np.float32)
    ceye = (c_hk.reshape(H * KH, 1, 1) * eye).astype(bf).reshape(CER, 128)
    shr = np.empty((SHR_ROWS, 1024), np.uint8)
    hdst = shr[R_HTS:R_HTS + HTR].reshape(KH, N, N)

    def quant(k):
        np.multiply(Hstack[k].T, 255.0, out=_QBUF[k])
        _QBUF[k] += 0.5
        hdst[k][:] = _QBUF[k]

    list(_POOL.map(quant, range(KH)))
    wqkvT = np.ascontiguousarray(Wqkv.T).astype(np.float32)
    wqkvT[:, :DIM] *= SCALE                            # fold q scaling
    u8row = lambda a: np.ascontiguousarray(a).view(np.uint8).reshape(-1, 1024)
    shr[R_WQKV:R_WPROJ] = u8row(wqkvT.astype(bf))
    shr[R_WPROJ:R_CEYE] = u8row(np.ascontiguousarray(Wproj.T).astype(bf))
    shr[R_CEYE:R_EYE] = u8row(ceye)
    shr[R_EYE:R_BPROJ] = u8row(eye.astype(bf))
    shr[R_BPROJ:R_HTS] = u8row(bproj.astype(np.float32).astype(bf)[None, :])
    shr[R_HTS + HTR:] = 0
    return shr


_QBUF = np.empty((KH, N, N), np.float32)

from concurrent.futures import ThreadPoolExecutor as _TPE
_POOL = _TPE(8)


def _cast_put_x(x, run):
    """Per-shard pipelined f32 -> fp8 cast + device_put: each worker casts
    its core's 2.1MB shard and immediately starts the async transfer, so
    the tunnel begins streaming ~5ms in instead of after the full cast."""
    import jax
    bufs = [None] * NCORES

    def do(c):
        xc = x[c * TOK:(c + 1) * TOK].astype(ml_dtypes.float8_e4m3)
        bufs[c] = jax.device_put(xc, run.devices[c])

    list(_POOL.map(do, range(NCORES)))
    return jax.make_array_from_single_device_arrays(
        (NCORES * TOK, DIM), run.spec, bufs)


def _make_runner(nc):
    """Persistent-jit runner for the bass_exec custom call (the axon/PJRT
    path), so warm calls skip tracing and output zero-buffers are created
    on device instead of being shipped from the host."""
    import jax
    import jax.numpy as jnp
    from jax.sharding import Mesh, PartitionSpec, NamedSharding
    from jax.experimental.shard_map import shard_map
    from concourse import mybir
    from concourse.bass2jax import (
        _bass_exec_p, partition_id_tensor, install_neuronx_cc_hook)

    install_neuronx_cc_hook()
    partition_name = nc.partition_id_tensor.name if nc.partition_id_tensor else None
    in_names, out_names, out_avals = [], [], []
    for alloc in nc.m.functions[0].allocations:
        if not isinstance(alloc, mybir.MemoryLocationSet):
            continue
        name = alloc.memorylocations[0].name
        if alloc.kind == "ExternalInput":
            if name != partition_name:
                in_names.append(name)
        elif alloc.kind == "ExternalOutput":
            out_names.append(name)
            out_avals.append(jax.core.ShapedArray(
                tuple(alloc.tensor_shape), mybir.dt.np(alloc.dtype)))
    n_params = len(in_names)
    n_outs = len(out_avals)
    all_names = in_names + out_names
    if partition_name is not None:
        all_names = all_names + [partition_name]
    donate = tuple(range(n_params, n_params + n_outs))

    def _body(*args):
        operands = list(args)
        if partition_name is not None:
            operands.append(partition_id_tensor())
        outs = _bass_exec_p.bind(
            *operands, out_avals=tuple(out_avals), in_names=tuple(all_names),
            out_names=tuple(out_names), lowering_input_output_aliases=(),
            sim_require_finite=True, sim_require_nnan=True, nc=nc)
        return tuple(outs)

    devices = jax.devices()[:NCORES]
    mesh = Mesh(np.asarray(devices), ("core",))
    spec = NamedSharding(mesh, PartitionSpec("core"))
    in_specs = (PartitionSpec("core"),) * (n_params + n_outs)
    out_specs = (PartitionSpec("core"),) * n_outs
    sharded = jax.jit(
        shard_map(_body, mesh=mesh, in_specs=in_specs, out_specs=out_specs,
                  check_rep=False),
        donate_argnums=donate, keep_unused=True)

    zero_shapes = [(NCORES * a.shape[0], *a.shape[1:]) for a in out_avals]
    zero_dtypes = [a.dtype for a in out_avals]
    zeros_fn = jax.jit(
        lambda: tuple(jnp.zeros(s, d) for s, d in zip(zero_shapes, zero_dtypes)),
        out_shardings=tuple(spec for _ in out_avals))

    prev = []

    def run(global_in: dict):
        ins = [global_in[name] for name in in_names]
        # donate the previous call's (already fetched) output buffers as the
        # custom call's result allocation; first call builds zeros on device
        zs = tuple(prev) if prev else zeros_fn()
        prev.clear()
        outs = sharded(*ins, *zs)
        res = {name: np.asarray(o) for name, o in zip(out_names, outs)}
        prev.extend(outs)
        return res

    run.spec = spec
    run.devices = devices
    return run


def kernel(**inputs):
    if "run" not in _CACHE:
        _CACHE["nc"] = _build()
        _CACHE["run"] = _make_runner(_CACHE["nc"])
    import jax
    run = _CACHE["run"]
    # cast + start the async x upload first; build the shared blob while the
    # 8.4MB of fp8 x streams over the tunnel
    x_dev = _cast_put_x(
        np.asarray(inputs["x"], np.float32).reshape(NCORES * TOK, DIM), run)
    shr = _prep_shared(
        np.asarray(inputs["Hstack"], np.float32),
        np.asarray(inputs["hop_logits_attn"], np.float32),
        np.asarray(inputs["rel_alpha"], np.float32),
        np.asarray(inputs["Wqkv"], np.float32),
        np.asarray(inputs["Wproj"], np.float32),
        np.asarray(inputs["bproj"], np.float32))
    outs = run({"xn": x_dev, "shr_in": shr})
    return _unpack_y(outs["y"])


def _unpack_y(yw):
    """Unpack [8*TOK, 388] u16 rows: 3 x 128-word planes holding 4 x 128
    12-bit value planes (w0=q0|(q1&15)<<12, w1=(q1>>4)|(q2&255)<<8,
    w2=(q2>>8)|q3<<4) + per-row f32 scale at words 384:386."""
    rows = yw.shape[0]
    y = np.empty((rows, DIM), np.float32)
    m = np.ascontiguousarray(yw[:, 384:386]).view(np.float32)  # [rows, 1]
    chunks = np.array_split(np.arange(rows), 8)

    def do(idx):
        sl = slice(idx[0], idx[-1] + 1)
        w0 = yw[sl, 0:128].astype(np.int32)
        w1 = yw[sl, 128:256].astype(np.int32)
        w2 = yw[sl, 256:384].astype(np.int32)
        y[sl, 0:128] = w0 & 4095
        y[sl, 128:256] = (w0 >> 12) | ((w1 & 255) << 4)
        y[sl, 256:384] = (w1 >> 8) | ((w2 & 15) << 8)
        y[sl, 384:512] = w2 >> 4
        y[sl] -= 2047.5
        y[sl] *= m[sl] / 2047.0

    list(_POOL.map(do, chunks))
    return y.reshape(B, N, DIM)


# revision 54
# speedup vs baseline: 1.3314x; 1.0319x over previous
"""Trainium2 Bass kernel for nn_Attention_xxc (dense transformer attention
with hop-distance bias). Data-parallel over batch: 8 cores x 2 batches.

Wire-traffic-minimized design: the warm end-to-end latency of this problem
is dominated by host<->device transfer over the axon tunnel (~50 MB/s), so
every shared tensor is shipped sharded 1/8-per-core and AllGathered on
device over NeuronLink; the hop-bias mixture  alpha_h * sum_k w_hk Hstack_k
is never materialized on the host - the PE folds it into the score matmuls
as  S.T = K^T Q + sum_k (c_hk I) @ Hstack_k.T  accumulated in PSUM.

Per-core layout (core c of 8):
  - xn [2048, 512] bf16: the core's own 2 batches, natural layout; the PE
    transposes it on device via identity matmuls.
  - shards (rows c/8) of: HTs flat [5120,1024] (Hstack_k transposed),
    wqkvT [512,1536] (q cols pre-scaled 1/sqrt(hd)), wprojT [512,512],
    ceye flat [5120,128] (40 scaled identities c_hk*I), eye128.
  - qkv: q,k TRANSPOSED ([outch, tok] bf16), v NATURAL with a ones column
    per head (65 cols/head) so the AV matmul also produces the softmax
    denominator in row 64.
  - output y [2048, 388] u16: each row's 512 values quantized to 12 bits
    against the row's absmax (computed on-device via a log-tree of
    tensor_tensor max/min - tensor_tensor_reduce faults the DVE here),
    packed 4 values -> 3 u16 words in contiguous 128-col planes, with the
    f32 row scale appended; the host unpacks and rescales.
Runner: persistent jax jit of the bass_exec custom call (no per-call
retrace); donated output buffers reuse the previous call's device arrays;
the fp8-x cast + device_put is pipelined per core shard so the tunnel
starts streaming immediately, overlapping the host-side blob build.
Validated vs reference (fixed seed): median rel err 0.0090,
scale-relative absmax 0.0111, under the 2e-2 gate.
"""
import sys

sys.path.insert(0, "/opt/trn_rl_repo")

import numpy as np
import ml_dtypes

B, N, DIM = 16, 1024, 512
H, HD, KH = 8, 64, 5
SCALE = HD ** -0.5
NCORES = 8
BPC = B // NCORES          # batches per core
TOK = BPC * N              # tokens per core = 2048
HTR = KH * N               # 5120 rows of flat transposed-Hstack
CER = H * KH * 128         # 5120 rows of flat scaled-identity stack

# shared-blob layout, in rows of 1024 bytes (= 512 bf16 / 1024 u8):
#   wqkv bf16 [512,1536] | wproj bf16 [512,512] | ceye bf16 [5120,128]
#   | eye bf16 [128,128] | bproj bf16 [512] | hts u8 [5120,1024] | pad
R_WQKV = 0
R_WPROJ = R_WQKV + 512 * 3
R_CEYE = R_WPROJ + 512
R_EYE = R_CEYE + CER // 4
R_BPROJ = R_EYE + 32
R_HTS = R_BPROJ + 1
SHR_ROWS = -(-(R_HTS + HTR) // 8) * 8    # pad to a multiple of 8 cores
YB = 388                   # packed y row: 384 u16 words + f32 scale + pad

_CACHE = {}


def _build():
    import concourse.bass as bass
    import concourse.bacc as bacc
    import concourse.mybir as mybir
    from concourse.tile import TileContext

    f32 = mybir.dt.float32
    bf16 = mybir.dt.bfloat16
    u8 = mybir.dt.uint8
    u16 = mybir.dt.uint16
    f8 = mybir.dt.float8e4
    AND = mybir.AluOpType.bitwise_and
    ORR = mybir.AluOpType.bitwise_or
    SHR = mybir.AluOpType.logical_shift_right
    SHL = mybir.AluOpType.logical_shift_left
    AMX = mybir.AluOpType.abs_max
    EXP = mybir.ActivationFunctionType.Exp
    MUL = mybir.AluOpType.mult
    ADD = mybir.AluOpType.add
    BYP = mybir.AluOpType.bypass
    RG = [list(range(NCORES))]

    nc = bacc.Bacc(num_devices=NCORES)
    xn = nc.declare_dram_parameter("xn", [TOK, DIM], f8, isOutput=False)
    shr_in = nc.declare_dram_parameter("shr_in", [SHR_ROWS // 8, 1024], u8, isOutput=False)
    y = nc.declare_dram_parameter("y", [TOK, YB], u16, isOutput=True)

    NT = TOK // 128            # 16 token tiles per core
    VW = H * (HD + 1)          # 520: v row width with ones col per head

    with TileContext(nc) as tc:
        with (
            tc.tile_pool(name="dram", bufs=1, space="DRAM") as DR,
            tc.tile_pool(name="qk", bufs=1) as QK,
            tc.tile_pool(name="vres", bufs=1) as VR,
            tc.tile_pool(name="wp", bufs=1) as WP,
            tc.tile_pool(name="outT", bufs=1) as OT,
            tc.tile_pool(name="const", bufs=1) as CONST,
        ):
            # ---------------- phase 0: AllGather the one shared blob ----------------
            bnc = DR.tile([SHR_ROWS // 8, 1024], u8, tag="b_shr", name="b_shr")
            shr_full = DR.tile([SHR_ROWS, 1024], u8, tag="g_shr", name="g_shr")
            nc.gpsimd.dma_start(bnc[:], shr_in[:])
            nc.gpsimd.collective_compute(
                "AllGather", BYP, replica_groups=RG,
                ins=[bnc.opt()], outs=[shr_full.opt()])

            eye_t = CONST.tile([128, 128], bf16, tag="eye", name="eye")
            nc.sync.dma_start(
                out=eye_t[:],
                in_=shr_full[R_EYE: R_EYE + 32, :].bitcast(bf16)
                .rearrange("a (b c) -> (a b) c", b=4))
            ones_t = CONST.tile([1, 128], bf16, tag="ones", name="ones")
            nc.vector.memset(ones_t[:], 1.0)
            ceye_t = CONST.tile([128, H * KH * 128], bf16, tag="ceye", name="ceye")
            for j in range(H * KH):
                nc.sync.dma_start(
                    out=ceye_t[:, j * 128:(j + 1) * 128],
                    in_=shr_full[R_CEYE + 32 * j: R_CEYE + 32 * (j + 1), :]
                    .bitcast(bf16).rearrange("a (b c) -> (a b) c", b=4))
            wp_t = [WP.tile([128, DIM], bf16, tag=f"wp{c}", name=f"wp{c}") for c in range(4)]
            for c in range(4):
                nc.sync.dma_start(
                    out=wp_t[c][:],
                    in_=shr_full[R_WPROJ + c * 128: R_WPROJ + (c + 1) * 128, :]
                    .bitcast(bf16))

            qk_t = [QK.tile([128, TOK], bf16, tag=f"qk{o}", name=f"qk{o}") for o in range(8)]
            v_t = [VR.tile([128, VW], bf16, tag=f"v{t}", name=f"v{t}") for t in range(NT)]
            oT_t = [OT.tile([128, N], bf16, tag=f"oT{b}_{c}", name=f"oT{b}_{c}")
                    for b in range(BPC) for c in range(4)]

            # broadcast bproj across 128 partitions: ones^T [128] x bproj [1,512]
            bpb_t = CONST.tile([128, DIM], f32, tag="bpb", name="bpb")
            bpr_t = CONST.tile([1, DIM], bf16, tag="bpr", name="bpr")
            nc.sync.dma_start(out=bpr_t[:],
                              in_=shr_full[R_BPROJ: R_BPROJ + 1, :].bitcast(bf16))

            # ---------------- phase 1: x transpose + qkv projections ----------------
            with (
                tc.tile_pool(name="xw", bufs=1) as XW,
                tc.tile_pool(name="ps1", bufs=4, space="PSUM") as PS1,
                tc.tile_pool(name="pst", bufs=4, space="PSUM") as PST,
            ):
                psb = PS1.tile([128, DIM], f32, tag="ps1", name="ps1")
                nc.tensor.matmul(psb[:], ones_t[:], bpr_t[:], start=True, stop=True)
                nc.vector.tensor_copy(bpb_t[:], psb[:])

                xn_t = [XW.tile([128, DIM], bf16, tag=f"xn{t}", name=f"xn{t}")
                        for t in range(NT)]
                for t in range(NT):
                    x8 = XW.tile([128, DIM], f8, tag=f"x8_{t}", name=f"x8_{t}")
                    nc.sync.dma_start(out=x8[:], in_=xn[t * 128:(t + 1) * 128, :])
                    nc.vector.tensor_copy(xn_t[t][:], x8[:])
                xT_t = [XW.tile([128, TOK], bf16, tag=f"x{c}", name=f"x{c}") for c in range(4)]
                for t in range(NT):
                    for c in range(4):
                        pst = PST.tile([128, 128], f32, tag="pst", name="pst")
                        nc.tensor.matmul(pst[:], xn_t[t][:, c * 128:(c + 1) * 128],
                                         eye_t[:], start=True, stop=True)
                        nc.vector.tensor_copy(xT_t[c][:, t * 128:(t + 1) * 128], pst[:])

                wq_t = [XW.tile([128, 3 * DIM], bf16, tag=f"w{c}", name=f"w{c}") for c in range(4)]
                for c in range(4):
                    for t in range(3):
                        nc.sync.dma_start(
                            out=wq_t[c][:, 512 * t:512 * (t + 1)],
                            in_=shr_full[R_WQKV + 384 * c + t:
                                         R_WQKV + 384 * (c + 1): 3, :].bitcast(bf16))

                # q,k transposed: qkvT[o_tile, tok] ; o tiles 0..7 cover q,k
                for o in range(8):
                    for t in range(4):           # tok chunks of 512
                        ps = PS1.tile([128, 512], f32, tag="ps1", name="ps1")
                        for c in range(4):
                            nc.tensor.matmul(
                                ps[:], wq_t[c][:, o * 128:(o + 1) * 128],
                                xT_t[c][:, t * 512:(t + 1) * 512],
                                start=(c == 0), stop=(c == 3))
                        nc.vector.tensor_copy(qk_t[o][:, t * 512:(t + 1) * 512], ps[:])
                # v natural: [tok_tile, vch] -> packed per head with ones col
                for t in range(NT):
                    ps = PS1.tile([128, 512], f32, tag="ps1", name="ps1")
                    for c in range(4):
                        nc.tensor.matmul(
                            ps[:], xT_t[c][:, t * 128:(t + 1) * 128],
                            wq_t[c][:, 2 * DIM:3 * DIM],
                            start=(c == 0), stop=(c == 3))
                    dst = v_t[t][:, 0:VW].rearrange("p (h s) -> p h s", s=HD + 1)
                    nc.vector.tensor_copy(
                        dst[:, :, 0:HD],
                        ps[:].rearrange("p (h s) -> p h s", s=HD))
                    nc.vector.memset(dst[:, :, HD:HD + 1], 1.0)

            # ---------------- phase 2: attention ----------------
            with (
                tc.tile_pool(name="htu", bufs=2) as HTU,
                tc.tile_pool(name="htp", bufs=3) as HTP,
                tc.tile_pool(name="pp", bufs=17) as PP,
                tc.tile_pool(name="nrm", bufs=4) as NRM,
                tc.tile_pool(name="ysb", bufs=2) as YSB,
                tc.tile_pool(name="pss", bufs=2, space="PSUM") as PSS,
                tc.tile_pool(name="pso", bufs=1, space="PSUM") as PSO,
                tc.tile_pool(name="psm", bufs=2, space="PSUM") as PSM,
            ):
                for h in range(H):
                    qt, po = qk_t[h // 2], (h % 2) * 64
                    kt = qk_t[4 + h // 2]
                    p_tiles = [[], []]
                    for mi in range(8):
                        hu = HTU.tile([128, KH * N], u8, tag="hu", name="hu")
                        for k in range(KH):
                            nc.sync.dma_start(
                                out=hu[:, k * N:(k + 1) * N],
                                in_=shr_full[R_HTS + k * N + mi * 128:
                                             R_HTS + k * N + (mi + 1) * 128, :])
                        ht = HTP.tile([128, KH * N], bf16, tag="ht", name="ht")
                        nc.vector.tensor_copy(ht[:], hu[:])
                        for b in range(BPC):
                            t0 = b * N
                            ps = PSS.tile([128, N], f32, tag="pss", name="pss")
                            for nchunk in range(2):
                                sl = slice(nchunk * 512, (nchunk + 1) * 512)
                                nc.tensor.matmul(
                                    ps[:, sl],
                                    kt[po:po + 64, t0 + mi * 128: t0 + (mi + 1) * 128],
                                    qt[po:po + 64, t0 + nchunk * 512: t0 + (nchunk + 1) * 512],
                                    start=True, stop=False)
                                for k in range(KH):
                                    ci = (h * KH + k) * 128
                                    nc.tensor.matmul(
                                        ps[:, sl],
                                        ceye_t[:, ci:ci + 128],
                                        ht[:, k * N + nchunk * 512:
                                           k * N + (nchunk + 1) * 512],
                                        start=False, stop=(k == KH - 1))
                            pt = PP.tile([128, N], bf16, tag="p", name="p")
                            nc.scalar.activation(pt[:], ps[:], EXP)
                            p_tiles[b].append(pt)
                    for b in range(BPC):
                        pso = PSO.tile([HD + 1, N], f32, tag="pso", name="pso")
                        for mi in range(8):
                            for nchunk in range(2):
                                sl = slice(nchunk * 512, (nchunk + 1) * 512)
                                nc.tensor.matmul(
                                    pso[:, sl],
                                    v_t[b * 8 + mi][:, h * (HD + 1):(h + 1) * (HD + 1)],
                                    p_tiles[b][mi][:, sl],
                                    start=(mi == 0), stop=(mi == 7))
                        # denominator -> broadcast -> reciprocal -> normalize
                        d_t = NRM.tile([1, N], bf16, tag="d", name="d")
                        nc.vector.tensor_copy(d_t[:], pso[64:65, :])
                        R_t = NRM.tile([64, N], f32, tag="R", name="R")
                        for nchunk in range(2):
                            sl = slice(nchunk * 512, (nchunk + 1) * 512)
                            psr = PSM.tile([64, 512], f32, tag="psm", name="psm")
                            nc.tensor.matmul(psr[:], ones_t[:, 0:64], d_t[:, sl],
                                             start=True, stop=True)
                            nc.vector.reciprocal(R_t[:, sl], psr[:])
                        nc.vector.tensor_tensor(
                            oT_t[b * 4 + h // 2][po:po + 64, :],
                            pso[0:64, :], R_t[:], MUL)
                # ---------------- phase 3: output projection ----------------
                # y + bias is quantized to 12 bits with a per-row dynamic
                # scale: row absmax m -> q = y*(2047/m)+2048 in [1,4095];
                # pairs (q0,q1) pack into 3 bytes; f32 m appended per row.
                for b in range(BPC):
                    for t in range(8):
                        psy = PSM.tile([128, 512], f32, tag="psm", name="psm")
                        for c in range(4):
                            nc.tensor.matmul(
                                psy[:],
                                oT_t[b * 4 + c][:, t * 128:(t + 1) * 128],
                                wp_t[c][:], start=(c == 0), stop=(c == 3))
                        # tensor_tensor_reduce faults the DVE at runtime on
                        # this stack; per-row max/min via log-tree instead
                        yt = YSB.tile([128, DIM], f32, tag="y", name="y")
                        m_t = NRM.tile([128, 1], f32, tag="m", name="m")
                        mn_t = NRM.tile([128, 1], f32, tag="mn", name="mn")
                        nc.vector.tensor_tensor(yt[:], psy[:], bpb_t[:], ADD)
                        ra = YSB.tile([128, 256], f32, tag="ra", name="ra")
                        rb = YSB.tile([128, 256], f32, tag="rb", name="rb")
                        for dst, rop in ((m_t, mybir.AluOpType.max),
                                         (mn_t, mybir.AluOpType.min)):
                            nc.vector.tensor_tensor(
                                ra[:, 0:256], yt[:, 0:256], yt[:, 256:512], rop)
                            cur, nxt, w = ra, rb, 128
                            while w >= 1:
                                nc.vector.tensor_tensor(
                                    nxt[:, 0:w], cur[:, 0:w], cur[:, w:2 * w], rop)
                                cur, nxt, w = nxt, cur, w // 2
                            nc.vector.tensor_copy(dst[:], cur[:, 0:1])
                        nc.vector.tensor_scalar_mul(mn_t[:], mn_t[:], -1.0)
                        nc.vector.tensor_tensor(
                            m_t[:], m_t[:], mn_t[:], mybir.AluOpType.max)
                        nc.vector.tensor_scalar_max(m_t[:], m_t[:], 1e-20)
                        s_t = NRM.tile([128, 1], f32, tag="s", name="s")
                        nc.vector.reciprocal(s_t[:], m_t[:])
                        nc.vector.tensor_scalar_mul(s_t[:], s_t[:], 2047.0)
                        qf = YSB.tile([128, DIM], f32, tag="qf", name="qf")
                        nc.vector.tensor_scalar(
                            qf[:], yt[:], s_t[:], 2048.0, MUL, ADD)
                        nc.vector.tensor_scalar(
                            qf[:], qf[:], 0.0, 4095.0,
                            mybir.AluOpType.max, mybir.AluOpType.min)
                        qu = YSB.tile([128, DIM], u16, tag="qu", name="qu")
                        nc.vector.tensor_copy(qu[:], qf[:])
                        # plane packing: q0..q3 = contiguous 128-col blocks;
                        # each (q0,q1,q2,q3) 4-tuple packs into 3 u16 words:
                        #   w0 = q0 | (q1&15)<<12
                        #   w1 = (q1>>4) | (q2&255)<<8
                        #   w2 = (q2>>8) | q3<<4
                        # all ops contiguous [128,128] u16->u16, no casts
                        q0, q1 = qu[:, 0:128], qu[:, 128:256]
                        q2b, q3 = qu[:, 256:384], qu[:, 384:512]
                        yw = YSB.tile([128, YB], u16, tag="yw", name="yw")
                        pa = YSB.tile([128, 128], u16, tag="pa", name="pa")
                        pb = YSB.tile([128, 128], u16, tag="pb", name="pb")
                        pc = YSB.tile([128, 128], u16, tag="pc", name="pc")
                        nc.vector.tensor_scalar(pa[:], q1, 15, 12, AND, SHL)
                        nc.vector.tensor_tensor(yw[:, 0:128], q0, pa[:], ORR)
                        nc.vector.tensor_scalar(pb[:], q2b, 255, 8, AND, SHL)
                        nc.vector.tensor_scalar(pc[:], q1, 4, None, SHR)
                        nc.vector.tensor_tensor(yw[:, 128:256], pc[:], pb[:], ORR)
                        nc.vector.tensor_scalar(pa[:], q2b, 8, None, SHR)
                        nc.vector.tensor_scalar(pb[:], q3, 4, None, SHL)
                        nc.vector.tensor_tensor(yw[:, 256:384], pa[:], pb[:], ORR)
                        nc.vector.tensor_copy(yw[:, 384:386], m_t[:].bitcast(u16))
                        nc.sync.dma_start(
                            out=y[b * N + t * 128: b * N + (t + 1) * 128, :],
                            in_=yw[:])
    nc.compile()
    return nc


def _prep_shared(Hstack, hop_logits_attn, rel_alpha, Wqkv, Wproj, bproj):
    """Build the shared blob [SHR_ROWS, 1024] u8 (concat-over-cores layout =
    the flat blob itself, so per-core shards are just row slices)."""
    bf = ml_dtypes.bfloat16
    lg = hop_logits_attn - hop_logits_attn.max(-1, keepdims=True)
    w = np.exp(lg)
    w /= w.sum(-1, keepdims=True)                      # [H, KH]
    # Hstack ships as uint8 (values in [0,1], quantization err ~ bf16's);
    # the 1/255 dequant scale is folded into the scaled identities.
    c_hk = (rel_alpha[:, None] * w).astype(np.float32) / 255.0  # [H, KH]
    eye = np.eye(128, dtype=np.float32)
    ceye = (c_hk.reshape(H * KH, 1, 1) * eye).astype(bf).reshape(CER, 128)
    shr = np.empty((SHR_ROWS, 1024), np.uint8)
    hdst = shr[R_HTS:R_HTS + HTR].reshape(KH, N, N)

    def quant(k):
        np.multiply(Hstack[k].T, 255.0, out=_QBUF[k])
        _QBUF[k] += 0.5
        hdst[k][:] = _QBUF[k]

    list(_POOL.map(quant, range(KH)))
    wqkvT = np.ascontiguousarray(Wqkv.T).astype(np.float32)
    wqkvT[:, :DIM] *= SCALE                            # fold q scaling
    u8row = lambda a: np.ascontiguousarray(a).view(np.uint8).reshape(-1, 1024)
    shr[R_WQKV:R_WPROJ] = u8row(wqkvT.astype(bf))
    shr[R_WPROJ:R_CEYE] = u8row(np.ascontiguousarray(Wproj.T).astype(bf))
    shr[R_CEYE:R_EYE] = u8row(ceye)
    shr[R_EYE:R_BPROJ] = u8row(eye.astype(bf))
    shr[R_BPROJ:R_HTS] = u8row(bproj.astype(np.float32).astype(bf)[None, :])
    shr[R_HTS + HTR:] = 0
    return shr


_QBUF = np.empty((KH, N, N), np.float32)

from concurrent.futures import ThreadPoolExecutor as _TPE
_POOL = _TPE(8)


def _cast_put_x(x, run):
    """Per-shard pipelined f32 -> fp8 cast + device_put: each worker casts
    its core's 2.1MB shard and immediately starts the async transfer, so
    the tunnel begins streaming ~5ms in instead of after the full cast."""
    import jax
    bufs = [None] * NCORES

    def do(c):
        xc = x[c * TOK:(c + 1) * TOK].astype(ml_dtypes.float8_e4m3)
        bufs[c] = jax.device_put(xc, run.devices[c])

    list(_POOL.map(do, range(NCORES)))
    return jax.make_array_from_single_device_arrays(
        (NCORES * TOK, DIM), run.spec, bufs)


def _make_runner(nc):
    """Persistent-jit runner for the bass_exec custom call (the axon/PJRT
    path), so warm calls skip tracing and output zero-buffers are created
    on device instead of being shipped from the host."""
    import jax
    import jax.numpy as jnp
    from jax.sharding import Mesh, PartitionSpec, NamedSharding
    from jax.experimental.shard_map import shard_map
    from concourse import mybir
    from concourse.bass2jax import (
        _bass_exec_p, partition_id_tensor, install_neuronx_cc_hook)

    install_neuronx_cc_hook()
    partition_name = nc.partition_id_tensor.name if nc.partition_id_tensor else None
    in_names, out_names, out_avals = [], [], []
    for alloc in nc.m.functions[0].allocations:
        if not isinstance(alloc, mybir.MemoryLocationSet):
            continue
        name = alloc.memorylocations[0].name
        if alloc.kind == "ExternalInput":
            if name != partition_name:
                in_names.append(name)
        elif alloc.kind == "ExternalOutput":
            out_names.append(name)
            out_avals.append(jax.core.ShapedArray(
                tuple(alloc.tensor_shape), mybir.dt.np(alloc.dtype)))
    n_params = len(in_names)
    n_outs = len(out_avals)
    all_names = in_names + out_names
    if partition_name is not None:
        all_names = all_names + [partition_name]
    donate = tuple(range(n_params, n_params + n_outs))

    def _body(*args):
        operands = list(args)
        if partition_name is not None:
            operands.append(partition_id_tensor())
        outs = _bass_exec_p.bind(
            *operands, out_avals=tuple(out_avals), in_names=tuple(all_names),
            out_names=tuple(out_names), lowering_input_output_aliases=(),
            sim_require_finite=True, sim_require_nnan=True, nc=nc)
        return tuple(outs)

    devices = jax.devices()[:NCORES]
    mesh = Mesh(np.asarray(devices), ("core",))
    spec = NamedSharding(mesh, PartitionSpec("core"))
    in_specs = (PartitionSpec("core"),) * (n_params + n_outs)
    out_specs = (PartitionSpec("core"),) * n_outs
    sharded = jax.jit(
        shard_map(_body, mesh=mesh, in_specs=in_specs, out_specs=out_specs,
                  check_rep=False),
        donate_argnums=donate, keep_unused=True)

    zero_shapes = [(NCORES * a.shape[0], *a.shape[1:]) for a in out_avals]
    zero_dtypes = [a.dtype for a in out_avals]
    zeros_fn = jax.jit(
        lambda: tuple(jnp.zeros(s, d) for s, d in zip(zero_shapes, zero_dtypes)),
        out_shardings=tuple(spec for _ in out_avals))

    prev = []

    def run(global_in: dict):
        ins = [global_in[name] for name in in_names]
        # donate the previous call's (already fetched) output buffers as the
        # custom call's result allocation; first call builds zeros on device
        zs = tuple(prev) if prev else zeros_fn()
        prev.clear()
        outs = sharded(*ins, *zs)
        res = {name: np.asarray(o) for name, o in zip(out_names, outs)}
        prev.extend(outs)
        return res

    run.spec = spec
    run.devices = devices
    return run


def kernel(**inputs):
    if "run" not in _CACHE:
        _CACHE["nc"] = _build()
        _CACHE["run"] = _make_runner(_CACHE["nc"])
    import jax
    run = _CACHE["run"]
    # cast + start the async x upload first; build the shared blob while the
    # 8.4MB of fp8 x streams over the tunnel
    x_dev = _cast_put_x(
        np.asarray(inputs["x"], np.float32).reshape(NCORES * TOK, DIM), run)
    shr = _prep_shared(
        np.asarray(inputs["Hstack"], np.float32),
        np.asarray(inputs["hop_logits_attn"], np.float32),
        np.asarray(inputs["rel_alpha"], np.float32),
        np.asarray(inputs["Wqkv"], np.float32),
        np.asarray(inputs["Wproj"], np.float32),
        np.asarray(inputs["bproj"], np.float32))
    outs = run({"xn": x_dev, "shr_in": shr})
    return _unpack_y(outs["y"])


def _unpack_y(yw):
    """Unpack [8*TOK, 388] u16 rows: 3 x 128-word planes holding 4 x 128
    12-bit value planes (w0=q0|(q1&15)<<12, w1=(q1>>4)|(q2&255)<<8,
    w2=(q2>>8)|q3<<4) + per-row f32 scale at words 384:386."""
    rows = yw.shape[0]
    y = np.empty((rows, DIM), np.float32)
    m = np.ascontiguousarray(yw[:, 384:386]).view(np.float32)  # [rows, 1]
    chunks = np.array_split(np.arange(rows), 8)

    def do(idx):
        sl = slice(idx[0], idx[-1] + 1)
        w0 = yw[sl, 0:128].astype(np.int32)
        w1 = yw[sl, 128:256].astype(np.int32)
        w2 = yw[sl, 256:384].astype(np.int32)
        y[sl, 0:128] = w0 & 4095
        y[sl, 128:256] = (w0 >> 12) | ((w1 & 255) << 4)
        y[sl, 256:384] = (w1 >> 8) | ((w2 & 15) << 8)
        y[sl, 384:512] = w2 >> 4
        y[sl] -= 2047.5
        y[sl] *= m[sl] / 2047.0

    list(_POOL.map(do, chunks))
    return y.reshape(B, N, DIM)


# revision 58
# speedup vs baseline: 1.4167x; 1.0641x over previous
"""Trainium2 Bass kernel for nn_Attention_xxc (dense transformer attention
with hop-distance bias). Data-parallel over batch: 8 cores x 2 batches.

Wire-traffic-minimized design: the warm end-to-end latency of this problem
is dominated by host<->device transfer over the axon tunnel (~50 MB/s), so
every shared tensor is shipped sharded 1/8-per-core and AllGathered on
device over NeuronLink; the hop-bias mixture  alpha_h * sum_k w_hk Hstack_k
is never materialized on the host - the PE folds it into the score matmuls
as  S.T = K^T Q + sum_k (c_hk I) @ Hstack_k.T  accumulated in PSUM.

Per-core layout (core c of 8):
  - xn [2048, 512] bf16: the core's own 2 batches, natural layout; the PE
    transposes it on device via identity matmuls.
  - shards (rows c/8) of: HTs flat [5120,1024] (Hstack_k transposed),
    wqkvT [512,1536] (q cols pre-scaled 1/sqrt(hd)), wprojT [512,512],
    ceye flat [5120,128] (40 scaled identities c_hk*I), eye128.
  - qkv: q,k TRANSPOSED ([outch, tok] bf16), v NATURAL with a ones column
    per head (65 cols/head) so the AV matmul also produces the softmax
    denominator in row 64.
  - output y [2048, 388] u16: each row's 512 values quantized to 12 bits
    against the row's absmax (computed on-device via a log-tree of
    tensor_tensor max/min - tensor_tensor_reduce faults the DVE here),
    packed 4 values -> 3 u16 words in contiguous 128-col planes, with the
    f32 row scale appended; the host unpacks and rescales.
Runner: persistent jax jit of the bass_exec custom call (no per-call
retrace); donated output buffers reuse the previous call's device arrays;
the fp8-x cast + device_put is pipelined per core shard so the tunnel
starts streaming immediately, overlapping the host-side blob build.
Validated vs reference (fixed seed): median rel err 0.0090,
scale-relative absmax 0.0111, under the 2e-2 gate.
"""
import sys

sys.path.insert(0, "/opt/trn_rl_repo")

import numpy as np
import ml_dtypes

B, N, DIM = 16, 1024, 512
H, HD, KH = 8, 64, 5
SCALE = HD ** -0.5
NCORES = 8
BPC = B // NCORES          # batches per core
TOK = BPC * N              # tokens per core = 2048
HTR = KH * N               # 5120 rows of flat transposed-Hstack
CER = H * KH * 128         # 5120 rows of flat scaled-identity stack

# shared-blob layout, in rows of 1024 bytes (= 512 bf16 / 1024 u8):
#   wqkv bf16 [512,1536] | wproj bf16 [512,512] | ceye bf16 [5120,128]
#   | eye bf16 [128,128] | bproj bf16 [512] | hts u8 [5120,1024] | pad
R_WQKV = 0
R_WPROJ = R_WQKV + 512 * 3
R_CEYE = R_WPROJ + 512
R_EYE = R_CEYE + CER // 4
R_BPROJ = R_EYE + 32
R_HTS = R_BPROJ + 1
SHR_ROWS = -(-(R_HTS + HTR) // 8) * 8    # pad to a multiple of 8 cores
YB = 388                   # packed y row: 384 u16 words + f32 scale + pad

_CACHE = {}


def _build():
    import concourse.bass as bass
    import concourse.bacc as bacc
    import concourse.mybir as mybir
    from concourse.tile import TileContext

    f32 = mybir.dt.float32
    bf16 = mybir.dt.bfloat16
    u8 = mybir.dt.uint8
    u16 = mybir.dt.uint16
    f8 = mybir.dt.float8e4
    AND = mybir.AluOpType.bitwise_and
    ORR = mybir.AluOpType.bitwise_or
    SHR = mybir.AluOpType.logical_shift_right
    SHL = mybir.AluOpType.logical_shift_left
    AMX = mybir.AluOpType.abs_max
    EXP = mybir.ActivationFunctionType.Exp
    MUL = mybir.AluOpType.mult
    ADD = mybir.AluOpType.add
    BYP = mybir.AluOpType.bypass
    RG = [list(range(NCORES))]

    nc = bacc.Bacc(num_devices=NCORES)
    xn = nc.declare_dram_parameter("xn", [TOK, DIM], f8, isOutput=False)
    shr_in = nc.declare_dram_parameter("shr_in", [SHR_ROWS // 8, 1024], u8, isOutput=False)
    y = nc.declare_dram_parameter("y", [TOK, YB], u16, isOutput=True)

    NT = TOK // 128            # 16 token tiles per core
    VW = H * (HD + 1)          # 520: v row width with ones col per head

    with TileContext(nc) as tc:
        with (
            tc.tile_pool(name="dram", bufs=1, space="DRAM") as DR,
            tc.tile_pool(name="qk", bufs=1) as QK,
            tc.tile_pool(name="vres", bufs=1) as VR,
            tc.tile_pool(name="wp", bufs=1) as WP,
            tc.tile_pool(name="outT", bufs=1) as OT,
            tc.tile_pool(name="const", bufs=1) as CONST,
        ):
            # ---------------- phase 0: AllGather the one shared blob ----------------
            bnc = DR.tile([SHR_ROWS // 8, 1024], u8, tag="b_shr", name="b_shr")
            shr_full = DR.tile([SHR_ROWS, 1024], u8, tag="g_shr", name="g_shr")
            nc.gpsimd.dma_start(bnc[:], shr_in[:])
            nc.gpsimd.collective_compute(
                "AllGather", BYP, replica_groups=RG,
                ins=[bnc.opt()], outs=[shr_full.opt()])

            eye_t = CONST.tile([128, 128], bf16, tag="eye", name="eye")
            nc.sync.dma_start(
                out=eye_t[:],
                in_=shr_full[R_EYE: R_EYE + 32, :].bitcast(bf16)
                .rearrange("a (b c) -> (a b) c", b=4))
            ones_t = CONST.tile([1, 128], bf16, tag="ones", name="ones")
            nc.vector.memset(ones_t[:], 1.0)
            ceye_t = CONST.tile([128, H * KH * 128], bf16, tag="ceye", name="ceye")
            for j in range(H * KH):
                nc.sync.dma_start(
                    out=ceye_t[:, j * 128:(j + 1) * 128],
                    in_=shr_full[R_CEYE + 32 * j: R_CEYE + 32 * (j + 1), :]
                    .bitcast(bf16).rearrange("a (b c) -> (a b) c", b=4))
            wp_t = [WP.tile([128, DIM], bf16, tag=f"wp{c}", name=f"wp{c}") for c in range(4)]
            for c in range(4):
                nc.sync.dma_start(
                    out=wp_t[c][:],
                    in_=shr_full[R_WPROJ + c * 128: R_WPROJ + (c + 1) * 128, :]
                    .bitcast(bf16))

            qk_t = [QK.tile([128, TOK], bf16, tag=f"qk{o}", name=f"qk{o}") for o in range(8)]
            v_t = [VR.tile([128, VW], bf16, tag=f"v{t}", name=f"v{t}") for t in range(NT)]
            oT_t = [OT.tile([128, N], bf16, tag=f"oT{b}_{c}", name=f"oT{b}_{c}")
                    for b in range(BPC) for c in range(4)]

            # broadcast bproj across 128 partitions: ones^T [128] x bproj [1,512]
            bpb_t = CONST.tile([128, DIM], f32, tag="bpb", name="bpb")
            bpr_t = CONST.tile([1, DIM], bf16, tag="bpr", name="bpr")
            nc.sync.dma_start(out=bpr_t[:],
                              in_=shr_full[R_BPROJ: R_BPROJ + 1, :].bitcast(bf16))

            # ---------------- phase 1: x transpose + qkv projections ----------------
            with (
                tc.tile_pool(name="xw", bufs=1) as XW,
                tc.tile_pool(name="ps1", bufs=4, space="PSUM") as PS1,
                tc.tile_pool(name="pst", bufs=4, space="PSUM") as PST,
            ):
                psb = PS1.tile([128, DIM], f32, tag="ps1", name="ps1")
                nc.tensor.matmul(psb[:], ones_t[:], bpr_t[:], start=True, stop=True)
                nc.vector.tensor_copy(bpb_t[:], psb[:])

                xn_t = [XW.tile([128, DIM], bf16, tag=f"xn{t}", name=f"xn{t}")
                        for t in range(NT)]
                for t in range(NT):
                    x8 = XW.tile([128, DIM], f8, tag=f"x8_{t}", name=f"x8_{t}")
                    nc.sync.dma_start(out=x8[:], in_=xn[t * 128:(t + 1) * 128, :])
                    nc.vector.tensor_copy(xn_t[t][:], x8[:])
                xT_t = [XW.tile([128, TOK], bf16, tag=f"x{c}", name=f"x{c}") for c in range(4)]
                for t in range(NT):
                    for c in range(4):
                        pst = PST.tile([128, 128], f32, tag="pst", name="pst")
                        nc.tensor.matmul(pst[:], xn_t[t][:, c * 128:(c + 1) * 128],
                                         eye_t[:], start=True, stop=True)
                        nc.vector.tensor_copy(xT_t[c][:, t * 128:(t + 1) * 128], pst[:])

                wq_t = [XW.tile([128, 3 * DIM], bf16, tag=f"w{c}", name=f"w{c}") for c in range(4)]
                for c in range(4):
                    for t in range(3):
                        nc.sync.dma_start(
                            out=wq_t[c][:, 512 * t:512 * (t + 1)],
                            in_=shr_full[R_WQKV + 384 * c + t:
                                         R_WQKV + 384 * (c + 1): 3, :].bitcast(bf16))

                # q,k transposed: qkvT[o_tile, tok] ; o tiles 0..7 cover q,k
                for o in range(8):
                    for t in range(4):           # tok chunks of 512
                        ps = PS1.tile([128, 512], f32, tag="ps1", name="ps1")
                        for c in range(4):
                            nc.tensor.matmul(
                                ps[:], wq_t[c][:, o * 128:(o + 1) * 128],
                                xT_t[c][:, t * 512:(t + 1) * 512],
                                start=(c == 0), stop=(c == 3))
                        nc.vector.tensor_copy(qk_t[o][:, t * 512:(t + 1) * 512], ps[:])
                # v natural: [tok_tile, vch] -> packed per head with ones col
                for t in range(NT):
                    ps = PS1.tile([128, 512], f32, tag="ps1", name="ps1")
                    for c in range(4):
                        nc.tensor.matmul(
                            ps[:], xT_t[c][:, t * 128:(t + 1) * 128],
                            wq_t[c][:, 2 * DIM:3 * DIM],
                            start=(c == 0), stop=(c == 3))
                    dst = v_t[t][:, 0:VW].rearrange("p (h s) -> p h s", s=HD + 1)
                    nc.vector.tensor_copy(
                        dst[:, :, 0:HD],
                        ps[:].rearrange("p (h s) -> p h s", s=HD))
                    nc.vector.memset(dst[:, :, HD:HD + 1], 1.0)

            # ---------------- phase 2: attention ----------------
            with (
                tc.tile_pool(name="htu", bufs=2) as HTU,
                tc.tile_pool(name="htp", bufs=3) as HTP,
                tc.tile_pool(name="pp", bufs=17) as PP,
                tc.tile_pool(name="nrm", bufs=4) as NRM,
                tc.tile_pool(name="ysb", bufs=2) as YSB,
                tc.tile_pool(name="pss", bufs=2, space="PSUM") as PSS,
                tc.tile_pool(name="pso", bufs=1, space="PSUM") as PSO,
                tc.tile_pool(name="psm", bufs=2, space="PSUM") as PSM,
            ):
                for h in range(H):
                    qt, po = qk_t[h // 2], (h % 2) * 64
                    kt = qk_t[4 + h // 2]
                    p_tiles = [[], []]
                    for mi in range(8):
                        hu = HTU.tile([128, KH * N], u8, tag="hu", name="hu")
                        for k in range(KH):
                            nc.sync.dma_start(
                                out=hu[:, k * N:(k + 1) * N],
                                in_=shr_full[R_HTS + k * N + mi * 128:
                                             R_HTS + k * N + (mi + 1) * 128, :])
                        ht = HTP.tile([128, KH * N], bf16, tag="ht", name="ht")
                        nc.vector.tensor_copy(ht[:], hu[:])
                        for b in range(BPC):
                            t0 = b * N
                            ps = PSS.tile([128, N], f32, tag="pss", name="pss")
                            for nchunk in range(2):
                                sl = slice(nchunk * 512, (nchunk + 1) * 512)
                                nc.tensor.matmul(
                                    ps[:, sl],
                                    kt[po:po + 64, t0 + mi * 128: t0 + (mi + 1) * 128],
                                    qt[po:po + 64, t0 + nchunk * 512: t0 + (nchunk + 1) * 512],
                                    start=True, stop=False)
                                for k in range(KH):
                                    ci = (h * KH + k) * 128
                                    nc.tensor.matmul(
                                        ps[:, sl],
                                        ceye_t[:, ci:ci + 128],
                                        ht[:, k * N + nchunk * 512:
                                           k * N + (nchunk + 1) * 512],
                                        start=False, stop=(k == KH - 1))
                            pt = PP.tile([128, N], bf16, tag="p", name="p")
                            nc.scalar.activation(pt[:], ps[:], EXP)
                            p_tiles[b].append(pt)
                    for b in range(BPC):
                        pso = PSO.tile([HD + 1, N], f32, tag="pso", name="pso")
                        for mi in range(8):
                            for nchunk in range(2):
                                sl = slice(nchunk * 512, (nchunk + 1) * 512)
                                nc.tensor.matmul(
                                    pso[:, sl],
                                    v_t[b * 8 + mi][:, h * (HD + 1):(h + 1) * (HD + 1)],
                                    p_tiles[b][mi][:, sl],
                                    start=(mi == 0), stop=(mi == 7))
                        # denominator -> broadcast -> reciprocal -> normalize
                        d_t = NRM.tile([1, N], bf16, tag="d", name="d")
                        nc.vector.tensor_copy(d_t[:], pso[64:65, :])
                        R_t = NRM.tile([64, N], f32, tag="R", name="R")
                        for nchunk in range(2):
                            sl = slice(nchunk * 512, (nchunk + 1) * 512)
                            psr = PSM.tile([64, 512], f32, tag="psm", name="psm")
                            nc.tensor.matmul(psr[:], ones_t[:, 0:64], d_t[:, sl],
                                             start=True, stop=True)
                            nc.vector.reciprocal(R_t[:, sl], psr[:])
                        nc.vector.tensor_tensor(
                            oT_t[b * 4 + h // 2][po:po + 64, :],
                            pso[0:64, :], R_t[:], MUL)
                # ---------------- phase 3: output projection ----------------
                # y + bias is quantized to 12 bits with a per-row dynamic
                # scale: row absmax m -> q = y*(2047/m)+2048 in [1,4095];
                # pairs (q0,q1) pack into 3 bytes; f32 m appended per row.
                for b in range(BPC):
                    for t in range(8):
                        psy = PSM.tile([128, 512], f32, tag="psm", name="psm")
                        for c in range(4):
                            nc.tensor.matmul(
                                psy[:],
                                oT_t[b * 4 + c][:, t * 128:(t + 1) * 128],
                                wp_t[c][:], start=(c == 0), stop=(c == 3))
                        # tensor_tensor_reduce faults the DVE at runtime on
                        # this stack; per-row max/min via log-tree instead
                        yt = YSB.tile([128, DIM], f32, tag="y", name="y")
                        m_t = NRM.tile([128, 1], f32, tag="m", name="m")
                        mn_t = NRM.tile([128, 1], f32, tag="mn", name="mn")
                        nc.vector.tensor_tensor(yt[:], psy[:], bpb_t[:], ADD)
                        ra = YSB.tile([128, 256], f32, tag="ra", name="ra")
                        rb = YSB.tile([128, 256], f32, tag="rb", name="rb")
                        for dst, rop in ((m_t, mybir.AluOpType.max),
                                         (mn_t, mybir.AluOpType.min)):
                            nc.vector.tensor_tensor(
                                ra[:, 0:256], yt[:, 0:256], yt[:, 256:512], rop)
                            cur, nxt, w = ra, rb, 128
                            while w >= 1:
                                nc.vector.tensor_tensor(
                                    nxt[:, 0:w], cur[:, 0:w], cur[:, w:2 * w], rop)
                                cur, nxt, w = nxt, cur, w // 2
                            nc.vector.tensor_copy(dst[:], cur[:, 0:1])
                        nc.vector.tensor_scalar_mul(mn_t[:], mn_t[:], -1.0)
                        nc.vector.tensor_tensor(
                            m_t[:], m_t[:], mn_t[:], mybir.AluOpType.max)
                        nc.vector.tensor_scalar_max(m_t[:], m_t[:], 1e-20)
                        s_t = NRM.tile([128, 1], f32, tag="s", name="s")
                        nc.vector.reciprocal(s_t[:], m_t[:])
                        nc.vector.tensor_scalar_mul(s_t[:], s_t[:], 2047.0)
                        qf = YSB.tile([128, DIM], f32, tag="qf", name="qf")
                        nc.vector.tensor_scalar(
                            qf[:], yt[:], s_t[:], 2048.0, MUL, ADD)
                        nc.vector.tensor_scalar(
                            qf[:], qf[:], 0.0, 4095.0,
                            mybir.AluOpType.max, mybir.AluOpType.min)
                        qu = YSB.tile([128, DIM], u16, tag="qu", name="qu")
                        nc.vector.tensor_copy(qu[:], qf[:])
                        # plane packing: q0..q3 = contiguous 128-col blocks;
                        # each (q0,q1,q2,q3) 4-tuple packs into 3 u16 words:
                        #   w0 = q0 | (q1&15)<<12
                        #   w1 = (q1>>4) | (q2&255)<<8
                        #   w2 = (q2>>8) | q3<<4
                        # all ops contiguous [128,128] u16->u16, no casts
                        q0, q1 = qu[:, 0:128], qu[:, 128:256]
                        q2b, q3 = qu[:, 256:384], qu[:, 384:512]
                        yw = YSB.tile([128, YB], u16, tag="yw", name="yw")
                        pa = YSB.tile([128, 128], u16, tag="pa", name="pa")
                        pb = YSB.tile([128, 128], u16, tag="pb", name="pb")
                        pc = YSB.tile([128, 128], u16, tag="pc", name="pc")
                        nc.vector.tensor_scalar(pa[:], q1, 15, 12, AND, SHL)
                        nc.vector.tensor_tensor(yw[:, 0:128], q0, pa[:], ORR)
                        nc.vector.tensor_scalar(pb[:], q2b, 255, 8, AND, SHL)
                        nc.vector.tensor_scalar(pc[:], q1, 4, None, SHR)
                        nc.vector.tensor_tensor(yw[:, 128:256], pc[:], pb[:], ORR)
                        nc.vector.tensor_scalar(pa[:], q2b, 8, None, SHR)
                        nc.vector.tensor_scalar(pb[:], q3, 4, None, SHL)
                        nc.vector.tensor_tensor(yw[:, 256:384], pa[:], pb[:], ORR)
                        nc.vector.tensor_copy(yw[:, 384:386], m_t[:].bitcast(u16))
                        nc.sync.dma_start(
                            out=y[b * N + t * 128: b * N + (t + 1) * 128, :],
                            in_=yw[:])
    nc.compile()
    return nc


def _prep_shared(Hstack, hop_logits_attn, rel_alpha, Wqkv, Wproj, bproj):
    """Build the shared blob [SHR_ROWS, 1024] u8 (concat-over-cores layout =
    the flat blob itself, so per-core shards are just row slices)."""
    bf = ml_dtypes.bfloat16
    lg = hop_logits_attn - hop_logits_attn.max(-1, keepdims=True)
    w = np.exp(lg)
    w /= w.sum(-1, keepdims=True)                      # [H, KH]
    # Hstack ships as uint8 (values in [0,1], quantization err ~ bf16's);
    # the 1/255 dequant scale is folded into the scaled identities.
    c_hk = (rel_alpha[:, None] * w).astype(np.float32) / 255.0  # [H, KH]
    eye = np.eye(128, dtype=np.float32)
    ceye = (c_hk.reshape(H * KH, 1, 1) * eye).astype(bf).reshape(CER, 128)
    shr = np.empty((SHR_ROWS, 1024), np.uint8)
    hdst = shr[R_HTS:R_HTS + HTR].reshape(KH, N, N)

    def quant(k):
        np.multiply(Hstack[k].T, 255.0, out=_QBUF[k])
        _QBUF[k] += 0.5
        hdst[k][:] = _QBUF[k]

    list(_POOL.map(quant, range(KH)))
    wqkvT = np.ascontiguousarray(Wqkv.T).astype(np.float32)
    wqkvT[:, :DIM] *= SCALE                            # fold q scaling
    u8row = lambda a: np.ascontiguousarray(a).view(np.uint8).reshape(-1, 1024)
    shr[R_WQKV:R_WPROJ] = u8row(wqkvT.astype(bf))
    shr[R_WPROJ:R_CEYE] = u8row(np.ascontiguousarray(Wproj.T).astype(bf))
    shr[R_CEYE:R_EYE] = u8row(ceye)
    shr[R_EYE:R_BPROJ] = u8row(eye.astype(bf))
    shr[R_BPROJ:R_HTS] = u8row(bproj.astype(np.float32).astype(bf)[None, :])
    shr[R_HTS + HTR:] = 0
    return shr


_QBUF = np.empty((KH, N, N), np.float32)

from concurrent.futures import ThreadPoolExecutor as _TPE
_POOL = _TPE(8)


def _cast_put_x(x, run):
    """Per-shard pipelined f32 -> fp8 cast + device_put: each worker casts
    its core's 2.1MB shard and immediately starts the async transfer, so
    the tunnel begins streaming ~5ms in instead of after the full cast."""
    import jax
    bufs = [None] * NCORES

    def do(c):
        xc = x[c * TOK:(c + 1) * TOK].astype(ml_dtypes.float8_e4m3)
        bufs[c] = jax.device_put(xc, run.devices[c])

    list(_POOL.map(do, range(NCORES)))
    return jax.make_array_from_single_device_arrays(
        (NCORES * TOK, DIM), run.spec, bufs)


def _put_sharded(a, run):
    """Per-shard threaded device_put of a host array (8 parallel streams)."""
    import jax
    rows = a.shape[0] // NCORES
    bufs = [None] * NCORES

    def do(c):
        bufs[c] = jax.device_put(a[c * rows:(c + 1) * rows], run.devices[c])

    list(_POOL.map(do, range(NCORES)))
    return jax.make_array_from_single_device_arrays(a.shape, run.spec, bufs)


def _make_runner(nc):
    """Persistent-jit runner for the bass_exec custom call (the axon/PJRT
    path), so warm calls skip tracing and output zero-buffers are created
    on device instead of being shipped from the host."""
    import jax
    import jax.numpy as jnp
    from jax.sharding import Mesh, PartitionSpec, NamedSharding
    from jax.experimental.shard_map import shard_map
    from concourse import mybir
    from concourse.bass2jax import (
        _bass_exec_p, partition_id_tensor, install_neuronx_cc_hook)

    install_neuronx_cc_hook()
    partition_name = nc.partition_id_tensor.name if nc.partition_id_tensor else None
    in_names, out_names, out_avals = [], [], []
    for alloc in nc.m.functions[0].allocations:
        if not isinstance(alloc, mybir.MemoryLocationSet):
            continue
        name = alloc.memorylocations[0].name
        if alloc.kind == "ExternalInput":
            if name != partition_name:
                in_names.append(name)
        elif alloc.kind == "ExternalOutput":
            out_names.append(name)
            out_avals.append(jax.core.ShapedArray(
                tuple(alloc.tensor_shape), mybir.dt.np(alloc.dtype)))
    n_params = len(in_names)
    n_outs = len(out_avals)
    all_names = in_names + out_names
    if partition_name is not None:
        all_names = all_names + [partition_name]
    donate = tuple(range(n_params, n_params + n_outs))

    def _body(*args):
        operands = list(args)
        if partition_name is not None:
            operands.append(partition_id_tensor())
        outs = _bass_exec_p.bind(
            *operands, out_avals=tuple(out_avals), in_names=tuple(all_names),
            out_names=tuple(out_names), lowering_input_output_aliases=(),
            sim_require_finite=True, sim_require_nnan=True, nc=nc)
        return tuple(outs)

    devices = jax.devices()[:NCORES]
    mesh = Mesh(np.asarray(devices), ("core",))
    spec = NamedSharding(mesh, PartitionSpec("core"))
    in_specs = (PartitionSpec("core"),) * (n_params + n_outs)
    out_specs = (PartitionSpec("core"),) * n_outs
    sharded = jax.jit(
        shard_map(_body, mesh=mesh, in_specs=in_specs, out_specs=out_specs,
                  check_rep=False),
        donate_argnums=donate, keep_unused=True)

    zero_shapes = [(NCORES * a.shape[0], *a.shape[1:]) for a in out_avals]
    zero_dtypes = [a.dtype for a in out_avals]
    zeros_fn = jax.jit(
        lambda: tuple(jnp.zeros(s, d) for s, d in zip(zero_shapes, zero_dtypes)),
        out_shardings=tuple(spec for _ in out_avals))

    prev = []

    def run(global_in: dict):
        ins = [global_in[name] for name in in_names]
        # donate the previous call's (already fetched) output buffers as the
        # custom call's result allocation; first call builds zeros on device
        zs = tuple(prev) if prev else zeros_fn()
        prev.clear()
        outs = sharded(*ins, *zs)
        prev.extend(outs)
        # caller must fetch these before invoking run() again (the next
        # call donates and overwrites them)
        return dict(zip(out_names, outs))

    run.spec = spec
    run.devices = devices
    return run


def kernel(**inputs):
    if "run" not in _CACHE:
        _CACHE["nc"] = _build()
        _CACHE["run"] = _make_runner(_CACHE["nc"])
    import jax
    run = _CACHE["run"]
    # cast + start the async x upload first; build the shared blob while the
    # 8.4MB of fp8 x streams over the tunnel
    x_dev = _cast_put_x(
        np.asarray(inputs["x"], np.float32).reshape(NCORES * TOK, DIM), run)
    shr = _prep_shared(
        np.asarray(inputs["Hstack"], np.float32),
        np.asarray(inputs["hop_logits_attn"], np.float32),
        np.asarray(inputs["rel_alpha"], np.float32),
        np.asarray(inputs["Wqkv"], np.float32),
        np.asarray(inputs["Wproj"], np.float32),
        np.asarray(inputs["bproj"], np.float32))
    shr_dev = _put_sharded(shr, run)
    outs = run({"xn": x_dev, "shr_in": shr_dev})
    return _unpack_y(outs["y"])


def _unpack_y(yw_dev):
    """Per-shard fetch + unpack, overlapped: each worker pulls its core's
    [2048, 388] u16 shard off the device and decodes it while other shards
    are still in flight. Rows: 3 x 128-word planes holding 4 x 128 12-bit
    value planes (w0=q0|(q1&15)<<12, w1=(q1>>4)|(q2&255)<<8,
    w2=(q2>>8)|q3<<4) + per-row f32 scale at words 384:386."""
    y = np.empty((NCORES * TOK, DIM), np.float32)

    def do(shard):
        r0 = shard.index[0].start or 0
        yw = np.asarray(shard.data)
        sl = slice(r0, r0 + yw.shape[0])
        m = np.ascontiguousarray(yw[:, 384:386]).view(np.float32)  # [rows,1]
        w0 = yw[:, 0:128].astype(np.int32)
        w1 = yw[:, 128:256].astype(np.int32)
        w2 = yw[:, 256:384].astype(np.int32)
        y[sl, 0:128] = w0 & 4095
        y[sl, 128:256] = (w0 >> 12) | ((w1 & 255) << 4)
        y[sl, 256:384] = (w1 >> 8) | ((w2 & 15) << 8)
        y[sl, 384:512] = w2 >> 4
        y[sl] -= 2047.5
        y[sl] *= m / 2047.0

    list(_POOL.map(do, yw_dev.addressable_shards))
    return y.reshape(B, N, DIM)
